# revision 74
# baseline (speedup 1.0000x reference)
"""Trainium2 Bass kernel for nn_CrossAttentionReranker (feature-major rewrite).

Reference math (seq_len==1 everywhere) collapses:
  - softmax over a size-1 axis == 1, so MHA(x_q, x_kv) == x_kv @ wv.T @ out_w.T
    -> folded on host (fp64) into a single [512,512] matmul per layer.
  - ln_w == 1, ln_b == 0 and all biases == 0 in setup_inputs() (asserted),
    so LayerNorm is pure normalize.

Device dataflow (per core, data-parallel over candidate rows):
  FEATURE-MAJOR activations: [128 feature-partitions x 512 rows] x 4 chunks
  ([128, 2048] bf16 tiles).  Candidates are pre-transposed to feature-major
  on the HOST, so no PE transposes are needed on device.  Matmuls use
  resident weight chunks as lhsT and activations as rhs.  Residual adds:
  chunks 0-1 via identity-matmul in the same PSUM accumulation group,
  chunks 2-3 fused into the DVE PSUM-evacuation (tensor_tensor add).
  LN stats: a single (1/512)*ones[128x128] PE matmul per chunk group
  computes sum, /512 and partition-broadcast at once.  z^2 for the variance
  is computed on the otherwise-idle Pool (gpsimd) engine.  Normalize = two
  [128,2048] DVE ops using stride-0 broadcast APs on mu/rstd.  Blocks run
  in software-pipelined waves of 3 with LN ops interleaved op-type-major
  across the wave.  Sigmoid deferred to one tail pass via a DRAM logits
  scratch (single ACT table switch).
"""

import os
import sys

import numpy as np
import ml_dtypes

N = 131072
D = 512
HID = 256
L = 2
P = 128
R = 512          # rows per block
NCORES = 8
EPS = 1e-5
WAVE = int(os.environ.get("KERNEL_WAVE", "3"))

BF16 = ml_dtypes.bfloat16
F8 = ml_dtypes.float8_e4m3
FP8 = os.environ.get("KERNEL_FP8", "1") == "1"
WS = 32.0  # fp8 weight pre-scale (exact power of two; unfolded via evac scale)

_cache: dict = {}
_runner_cache: dict = {}


def _chunk(w: np.ndarray) -> np.ndarray:
    """[K, M] (K multiple of 128) -> [128, (K//128)*M], K-chunk-major on free dim."""
    k, m = w.shape
    assert k % P == 0
    return np.ascontiguousarray(
        w.reshape(k // P, P, m).transpose(1, 0, 2).reshape(P, (k // P) * m)
    )


def _prep_host(inputs):
    """Fold weights on host (fp64), cast to bf16, pre-chunk for lhsT layout."""
    f8 = np.float64
    assert np.all(np.asarray(inputs["ln_w"]) == 1.0), "kernel assumes ln_w == 1"
    assert not np.any(np.asarray(inputs["ln_b"])), "kernel assumes ln_b == 0"
    for k in ("attn_in_b", "attn_out_b", "ffn_b1", "ffn_b2", "head_b1", "head_b2"):
        assert not np.any(np.asarray(inputs[k])), f"kernel assumes {k} == 0"

    wdt = F8 if FP8 else BF16
    wsc = WS if FP8 else 1.0
    arrs = {}
    # wa and w2 feed LayerNorms whose residuals are themselves LN outputs
    # (feature-sum 0), so centering their OUTPUT dim on host makes every such
    # LN input exactly zero-mean: the device then skips mean subtraction
    # entirely (LN is invariant to the uniform shift removed here).
    for i in range(L):
        wv = np.asarray(inputs["attn_in_w"])[i][2 * D :].astype(f8)  # [D, D]
        ow = np.asarray(inputs["attn_out_w"])[i].astype(f8)          # [D, D]
        wa = wv.T @ ow.T                                             # x @ wa == mha(x)
        wa = wa - wa.mean(axis=1, keepdims=True)
        arrs[f"wa{i}"] = (_chunk(wa) * wsc).astype(wdt)              # [128, 4*512]
        w1 = np.asarray(inputs["ffn_w1"])[i].T.astype(f8)            # [512, 256]
        arrs[f"w1_{i}"] = (_chunk(w1) * wsc).astype(wdt)             # [128, 4*256]
        w2 = np.asarray(inputs["ffn_w2"])[i].T.astype(f8)            # [256, 512]
        w2 = w2 - w2.mean(axis=1, keepdims=True)
        arrs[f"w2_{i}"] = (_chunk(w2) * wsc).astype(wdt)             # [128, 2*512]
    arrs["h1"] = _chunk(np.asarray(inputs["head_w1"]).T.astype(f8)).astype(BF16)
    arrs["h2"] = _chunk(np.asarray(inputs["head_w2"]).T.astype(f8)).astype(BF16)
    # q0 appears only as an LN residual; centering it is exact (LN shift
    # invariance) and keeps that LN input zero-mean too.
    q0 = np.asarray(inputs["query_embedding"]).astype(f8).reshape(D)
    q0 = (q0 - q0.mean()).astype(np.float32)
    # feature-major q0, replicated along the row (free) dim: chunk j columns
    # all equal q0[j*128:(j+1)*128]
    q0T = np.concatenate(
        [np.broadcast_to(q0[j * P : (j + 1) * P][:, None], (P, R)) for j in range(4)],
        axis=1,
    )
    arrs["q0T"] = np.ascontiguousarray(q0T).astype(F8 if FP8 else BF16)  # [128, 4*512]
    # identity (scaled by WS under fp8 so PE residual adds match the psum scale)
    arrs["identb"] = (np.eye(P, dtype=np.float32) * wsc).astype(wdt)
    # all-(1/512) matrix: ones^T/D @ x = column means, broadcast to all partitions
    arrs["omat"] = np.full((P, P), 1.0 / D, dtype=np.float32).astype(BF16)
    return arrs


def _prep_cand(cand: np.ndarray) -> np.ndarray:
    """[N, D] row-major -> per-core feature-major chunked [NCORES, 128, nblk*4*R].

    candT[core, p, (b*4 + c)*R + j] == cand[core*rows + b*R + j, c*128 + p]
    """
    n = cand.shape[0]
    rows = n // NCORES
    nblk = rows // R
    x = np.asarray(cand).astype(F8 if FP8 else BF16).reshape(NCORES, nblk, R, 4, P)
    return np.ascontiguousarray(
        x.transpose(0, 4, 1, 3, 2).reshape(NCORES, P, nblk * 4 * R)
    )


def _build_program(rows_per_core: int):
    """Trace + schedule + compile the Bass program for one core (SPMD)."""
    import concourse.bass as bass
    import concourse.mybir as mybir
    import concourse.tile as tile
    from concourse import bacc
    from concourse.bass import ts

    dt = mybir.dt
    alu = mybir.AluOpType
    act_fn = mybir.ActivationFunctionType
    nblk = rows_per_core // R
    assert rows_per_core % R == 0

    nc = bacc.Bacc(
        "TRN2", target_bir_lowering=False, debug=False, num_devices=NCORES
    )

    wdt = dt.float8e4 if FP8 else dt.bfloat16
    adt = dt.float8e4 if FP8 else dt.bfloat16
    candT = nc.dram_tensor(
        "candT", [P, nblk * 4 * R], adt, kind="ExternalInput"
    )
    dr = {}
    for i in range(L):
        dr[f"wa{i}"] = nc.dram_tensor(f"wa{i}", [P, 4 * D], wdt, kind="ExternalInput")
        dr[f"w1_{i}"] = nc.dram_tensor(f"w1_{i}", [P, 4 * HID], wdt, kind="ExternalInput")
        dr[f"w2_{i}"] = nc.dram_tensor(f"w2_{i}", [P, 2 * D], wdt, kind="ExternalInput")
    dr["h1"] = nc.dram_tensor("h1", [P, 8 * HID], dt.bfloat16, kind="ExternalInput")
    dr["h2"] = nc.dram_tensor("h2", [P, 2], dt.bfloat16, kind="ExternalInput")
    dr["q0T"] = nc.dram_tensor("q0T", [P, 4 * R], adt, kind="ExternalInput")
    dr["identb"] = nc.dram_tensor("identb", [P, P], wdt, kind="ExternalInput")
    dr["omat"] = nc.dram_tensor("omat", [P, P], dt.bfloat16, kind="ExternalInput")
    lgs = nc.dram_tensor("lgs", [nblk * R], dt.float32, kind="Internal")
    scores = nc.dram_tensor("scores", [rows_per_core, 1], dt.float32, kind="ExternalOutput")

    from contextlib import ExitStack

    with tile.TileContext(nc) as tc, ExitStack() as ctx:
        const = ctx.enter_context(tc.tile_pool(name="const", bufs=1))

        def load_const(name, shape, dtype):
            t = const.tile(shape, dtype, tag=f"const_{name}")
            nc.sync.dma_start(t[:], dr[name].ap())
            return t

        wsb = []
        for i in range(L):
            wsb.append(
                (
                    load_const(f"wa{i}", [P, 4 * D], wdt),
                    load_const(f"w1_{i}", [P, 4 * HID], wdt),
                    load_const(f"w2_{i}", [P, 2 * D], wdt),
                )
            )
        h1sb = load_const("h1", [P, 8 * HID], dt.bfloat16)
        h2sb = load_const("h2", [P, 2], dt.bfloat16)
        q0T = load_const("q0T", [P, 4 * R], adt)
        identb = load_const("identb", [P, P], wdt)
        omat = load_const("omat", [P, P], dt.bfloat16)
        eps_t = const.tile([P, 1], dt.float32, tag="eps")
        nc.gpsimd.memset(eps_t[:], float(EPS))

        xp = ctx.enter_context(tc.tile_pool(name="xp", bufs=4))
        zp = ctx.enter_context(tc.tile_pool(name="zp", bufs=4))
        sqp = ctx.enter_context(tc.tile_pool(name="sqp", bufs=3))
        stp = ctx.enter_context(tc.tile_pool(name="stp", bufs=5))
        ap_ = ctx.enter_context(tc.tile_pool(name="ap", bufs=18))
        hp = ctx.enter_context(tc.tile_pool(name="hp", bufs=4))
        fin = ctx.enter_context(tc.tile_pool(name="fin", bufs=1))
        lout = ctx.enter_context(tc.tile_pool(name="lout", bufs=2))
        py = ctx.enter_context(tc.tile_pool(name="py", bufs=5, space="PSUM"))
        pst = ctx.enter_context(tc.tile_pool(name="pst", bufs=2, space="PSUM"))
        plg = ctx.enter_context(tc.tile_pool(name="plg", bufs=1, space="PSUM"))

        # residual chunks 0..PE_RES-1 added via identity matmul in PSUM;
        # chunks PE_RES..3 added by DVE during PSUM evacuation.
        PE_RES = int(os.environ.get("KERNEL_PE_RES", "2"))

        def pair2(ap_in, stride, num):
            """[128, X] AP -> [128, 2, num] AP pairing two adjacent k-tiles."""
            return bass.AP(
                ap_in.tensor, ap_in.offset, [ap_in.ap[0], [stride, 2], [1, num]]
            )

        def mm_stage(w_sb, rhs, nk, nfo, resid=None, force_bf16=False):
            """y^T chunks: out[fo] = sum_k w[k,fo-block]^T @ rhs[k] (+ resid[fo]).

            w_sb: [128, nk*nfo*128] chunked lhsT; rhs(k) -> [128, R] AP;
            resid(fo) -> [128, R] AP or None (PE-added only for fo < PE_RES).
            Under fp8, k-chunks are processed in pairs with DoubleRow perf
            mode (256-deep contraction per matmul at double rate).
            Returns list of PSUM tiles.
            """
            m = nfo * P
            ys = []
            for fo in range(nfo):
                pe_resid = resid is not None and fo < PE_RES
                y = py.tile([P, R], dt.float32, tag="y")
                if FP8 and not force_bf16:
                    assert nk % 2 == 0
                    for t in range(nk // 2):
                        w_ap = w_sb[:, 2 * t * m + fo * P : 2 * t * m + fo * P + P]
                        r_ap = rhs(2 * t)
                        nc.tensor.matmul(
                            y[:, :],
                            pair2(w_ap, m, P),
                            pair2(r_ap, R, R),
                            start=(t == 0),
                            stop=(t == nk // 2 - 1 and not pe_resid),
                            perf_mode=mybir.MatmulPerfMode.DoubleRow,
                        )
                else:
                    for k in range(nk):
                        nc.tensor.matmul(
                            y[:, :],
                            w_sb[:, k * m + fo * P : k * m + (fo + 1) * P],
                            rhs(k),
                            start=(k == 0),
                            stop=(k == nk - 1 and not pe_resid),
                        )
                if pe_resid:
                    nc.tensor.matmul(
                        y[:, :], identb[:], resid(fo), start=False, stop=True
                    )
                ys.append(y)
            return ys

        def ln_multi(ys_list, resid_list, out_dtype=None, twin=False,
                     need_mean=False):
            """LN for a wave of in-flight blocks; ops interleaved op-type-major
            across blocks so no engine stream has long dependent runs.

            Chunks >= PE_RES get their residual added during the DVE
            evacuation; chunks < PE_RES were already summed in PSUM.

            need_mean=False: the LN input is zero-mean by construction
            (host-centered weights + LN-output residuals), so the variance is
            just the second moment and no mean subtraction happens at all.
            need_mean=True (only layer-0 c-side, whose residual is the raw
            candidate embedding): full mean/variance path."""
            n = len(ys_list)
            Z, SQ, SBZ, SBQ, ME, MU2, VEPS, STD, RSTD, A = (
                [None] * n for _ in range(10)
            )
            for j, ys in enumerate(ys_list):
                _lvl = int(os.environ.get("KERNEL_LN_LEVEL", "3"))
                zpool = ap_ if _lvl < 3 else zp
                Z[j] = zpool.tile(
                    [P, 4 * R], adt if _lvl < 3 else dt.bfloat16,
                    name=f"z{j}", tag="z",
                )
                for c in range(4):
                    if c < PE_RES:
                        # psum holds WS*(W x + resid); evac unscales
                        nc.scalar.activation(
                            out=Z[j][:, ts(c, R)], in_=ys[c][:], func=act_fn.Copy,
                            scale=1.0 / WS if FP8 else 1.0,
                        )
                    elif FP8:
                        nc.vector.scalar_tensor_tensor(
                            out=Z[j][:, ts(c, R)], in0=ys[c][:],
                            scalar=1.0 / WS, in1=resid_list[j](c),
                            op0=alu.mult, op1=alu.add,
                        )
                    else:
                        nc.vector.tensor_tensor(
                            out=Z[j][:, ts(c, R)], in0=ys[c][:],
                            in1=resid_list[j](c), op=alu.add,
                        )
            LVL = int(os.environ.get("KERNEL_LN_LEVEL", "3"))
            NOSQ = os.environ.get("KERNEL_NOSQ", "0") == "1"
            NOSTATS = os.environ.get("KERNEL_NOSTATS", "0") == "1"
            NORECIP = os.environ.get("KERNEL_NORECIP", "0") == "1"
            NB = os.environ.get("KERNEL_NB", "0") == "1"
            if LVL == 0:
                return (Z, Z) if twin else Z
            # z^2: chunks 0-1 on Act; chunks 2-3 on Pool (gpsimd) if enabled
            # (cost model says Pool is cheap, HW measurements disagree).
            SQ_POOL = os.environ.get("KERNEL_SQ_POOL", "0") == "1"
            for j in range(n):
                SQ[j] = sqp.tile([P, 4 * R], dt.bfloat16, name=f"zsq{j}", tag="zsq")
                if NOSQ:
                    SQ[j] = Z[j]
                    continue
                nc.scalar.activation(
                    out=SQ[j][:, 0 : 2 * R], in_=Z[j][:, 0 : 2 * R],
                    func=act_fn.Square,
                )
                if SQ_POOL:
                    nc.gpsimd.tensor_tensor(
                        out=SQ[j][:, 2 * R : 4 * R], in0=Z[j][:, 2 * R : 4 * R],
                        in1=Z[j][:, 2 * R : 4 * R], op=alu.mult,
                    )
                else:
                    nc.scalar.activation(
                        out=SQ[j][:, 2 * R : 4 * R], in_=Z[j][:, 2 * R : 4 * R],
                        func=act_fn.Square,
                    )
            # stats: sum + /512 + partition-broadcast in one PE op per chunk:
            # SBQ = (ones/512)^T @ sq-chunks, accumulated -> every partition
            # holds the column mean of z^2 (the variance, since z is
            # zero-mean unless need_mean).  The sum-z pass exists only for
            # the need_mean LN.
            if need_mean:
                for j in range(n):
                    SBZ[j] = py.tile([P, R], dt.float32, name=f"sbz{j}", tag="y")
                    for c in range(4):
                        nc.tensor.matmul(
                            SBZ[j][:], omat[:], Z[j][:, ts(c, R)],
                            start=(c == 0), stop=(c == 3),
                        )
            for j in range(n):
                SBQ[j] = pst.tile([P, R], dt.float32, name=f"sbq{j}", tag="sb")
                for c in range(4):
                    nc.tensor.matmul(
                        SBQ[j][:], omat[:], SQ[j][:, ts(c, R)],
                        start=(c == 0), stop=(c == 3),
                    )
            if need_mean:
                # ME and MU2 depend only on SBZ (done before the sumsq
                # matmuls), so both run during the sumsq pass.
                for j in range(n):
                    ME[j] = stp.tile([P, R], dt.bfloat16, name=f"me{j}", tag="me")
                    nc.scalar.activation(
                        out=ME[j][:], in_=SBZ[j][:], func=act_fn.Copy
                    )
            if LVL == 1:
                return (Z, Z) if twin else Z
            if need_mean:
                for j in range(n):
                    MU2[j] = stp.tile([P, R], dt.bfloat16, name=f"mu2{j}", tag="mu2")
                    nc.vector.tensor_tensor(
                        out=MU2[j][:], in0=ME[j][:], in1=ME[j][:], op=alu.mult
                    )
                for j in range(n):
                    VEPS[j] = stp.tile(
                        [P, R], dt.bfloat16, name=f"veps{j}", tag="veps"
                    )
                    nc.vector.scalar_tensor_tensor(
                        out=VEPS[j][:], in0=SBQ[j][:], scalar=1.0,
                        in1=MU2[j][:], op0=alu.bypass, op1=alu.subtract,
                    )
            for j in range(n):
                STD[j] = stp.tile([P, R], dt.float32, name=f"std{j}", tag="std")
                nc.scalar.activation(
                    out=STD[j][:],
                    in_=VEPS[j][:] if need_mean else SBQ[j][:],
                    func=act_fn.Sqrt, bias=eps_t[:],
                )

            if LVL == 2:
                return (Z, Z) if twin else Z

            for j in range(n):
                A[j] = ap_.tile(
                    [P, 4 * R], out_dtype or adt, name=f"a{j}", tag="a"
                )
            # normalize per chunk with plain 2-level stride-1 APs (stride-0
            # broadcast APs measured ~3x slower on HW); the [128,R] stat tile
            # already holds the per-column value on every partition.  Under
            # fp8 the subtract lands in the (now-free) bf16 SQ tile so the
            # activation is only fp8-quantized once, at the final multiply.
            if need_mean:
                AS = [SQ[j] if FP8 else A[j] for j in range(n)]
                for c in range(4):
                    for j in range(n):
                        nc.vector.tensor_tensor(
                            out=AS[j][:, ts(c, R)],
                            in0=Z[j][:, ts(c, R)],
                            in1=ME[j][:],
                            op=alu.subtract,
                        )
            else:
                AS = Z
            for j in range(n):
                # fp32 1/std via the fast custom-DVE op (InstReciprocal
                # measured ~4.5x slower on HW than the cost model claims),
                # then a bf16 convert on Act so the multiplies stay 2-byte.
                RSTDF = stp.tile([P, R], dt.float32, name=f"rstdf{j}", tag="rstdf")
                nc.vector.reciprocal_approx_fast(out=RSTDF[:], in_=STD[j][:])
                RSTD[j] = stp.tile([P, R], dt.bfloat16, name=f"rstd{j}", tag="rstd")
                nc.scalar.activation(
                    out=RSTD[j][:], in_=RSTDF[:], func=act_fn.Copy
                )
            for c in range(4):
                for j in range(n):
                    nc.vector.tensor_tensor(
                        out=A[j][:, ts(c, R)],
                        in0=AS[j][:, ts(c, R)],
                        in1=RSTD[j][:],
                        op=alu.mult,
                    )
            if not twin:
                return A
            # bf16 twin of the normalized output (for the bf16 head path)
            AB = [None] * n
            for j in range(n):
                AB[j] = ap_.tile(
                    [P, 4 * R], dt.bfloat16, name=f"ab{j}", tag="ab", bufs=WAVE + 1
                )
            for c in range(4):
                for j in range(n):
                    nc.vector.tensor_tensor(
                        out=AB[j][:, ts(c, R)],
                        in0=AS[j][:, ts(c, R)],
                        in1=RSTD[j][:],
                        op=alu.mult,
                    )
            return A, AB

        def input_stage(b):
            cT = xp.tile([P, 4 * R], adt, tag="cT")
            nc.sync.dma_start(cT[:], candT.ap()[:, b * 4 * R : (b + 1) * 4 * R])
            return cT

        def relu_multi(hps_list, out_dtype, tag, unscale=True):
            # relu is positively homogeneous: Relu(psum/WS) unscales exactly
            hs = []
            for j, hps in enumerate(hps_list):
                h = hp.tile([P, 2 * R], out_dtype, name=f"h{j}", tag=tag)
                for fo in range(2):
                    nc.scalar.activation(
                        out=h[:, ts(fo, R)], in_=hps[fo][:], func=act_fn.Relu,
                        scale=1.0 / WS if (FP8 and unscale) else 1.0,
                    )
                hs.append(h)
            return hs

        for w0 in range(0, nblk, WAVE):
            wb = list(range(w0, min(w0 + WAVE, nblk)))
            st = [{"b": b} for b in wb]
            for S in st:
                S["cT"] = input_stage(S["b"])
                S["q"], S["c"] = q0T, S["cT"]
            for i in range(L):
                wa, w1, w2 = wsb[i]
                for S in st:
                    S["y"] = mm_stage(
                        wa, lambda k, S=S: S["c"][:, ts(k, R)], 4, 4,
                        resid=lambda fo, S=S: S["q"][:, ts(fo, R)],
                    )
                a1s = ln_multi(
                    [S["y"] for S in st],
                    [lambda c, S=S: S["q"][:, ts(c, R)] for S in st],
                )
                for S, a1 in zip(st, a1s):
                    S["a1"] = a1
                    S["hps"] = mm_stage(w1, lambda k, a1=a1: a1[:, ts(k, R)], 4, 2)
                hs = relu_multi([S["hps"] for S in st], adt, "h")
                for S, h in zip(st, hs):
                    S["y"] = mm_stage(
                        w2, lambda k, h=h: h[:, ts(k, R)], 2, 4,
                        resid=lambda fo, S=S: S["a1"][:, ts(fo, R)],
                    )
                if FP8 and i == L - 1:
                    # bf16 twin of the final q for the bf16 head path
                    a2s, a2bs = ln_multi(
                        [S["y"] for S in st],
                        [lambda c, S=S: S["a1"][:, ts(c, R)] for S in st],
                        twin=True,
                    )
                else:
                    a2s = ln_multi(
                        [S["y"] for S in st],
                        [lambda c, S=S: S["a1"][:, ts(c, R)] for S in st],
                    )
                    a2bs = a2s
                for S, a2, a2b in zip(st, a2s, a2bs):
                    S["a2"] = a2
                    S["qh"] = a2b
                    S["y"] = mm_stage(
                        wa, lambda k, a2=a2: a2[:, ts(k, R)], 4, 4,
                        resid=lambda fo, S=S: S["c"][:, ts(fo, R)],
                    )
                a3s = ln_multi(
                    [S["y"] for S in st],
                    [lambda c, S=S: S["c"][:, ts(c, R)] for S in st],
                    need_mean=(i == 0),
                )
                for S, a3 in zip(st, a3s):
                    S["a3"] = a3
                    S["hps"] = mm_stage(w1, lambda k, a3=a3: a3[:, ts(k, R)], 4, 2)
                hs = relu_multi([S["hps"] for S in st], adt, "h")
                for S, h in zip(st, hs):
                    S["y"] = mm_stage(
                        w2, lambda k, h=h: h[:, ts(k, R)], 2, 4,
                        resid=lambda fo, S=S: S["a3"][:, ts(fo, R)],
                    )
                a4s = ln_multi(
                    [S["y"] for S in st],
                    [lambda c, S=S: S["a3"][:, ts(c, R)] for S in st],
                    out_dtype=dt.bfloat16 if i == L - 1 else None,
                )
                for S, a4 in zip(st, a4s):
                    S["q"], S["c"] = S["a2"], a4

            # head: combined = [q | c] -> HID -> 1, fully bf16 (head errors
            # reach the logits unwashed by any later LN)
            for S in st:
                S["hps"] = mm_stage(
                    h1sb,
                    lambda k, S=S: (
                        S["qh"][:, ts(k, R)] if k < 4 else S["c"][:, ts(k - 4, R)]
                    ),
                    8, 2, force_bf16=True,
                )
            hhs = relu_multi([S["hps"] for S in st], dt.bfloat16, "hh", unscale=False)
            for S, hh in zip(st, hhs):
                lg = plg.tile([1, R], dt.float32, tag="lg")
                for k in range(2):
                    nc.tensor.matmul(
                        lg[:, :], h2sb[:, k : k + 1], hh[:, ts(k, R)],
                        start=(k == 0), stop=(k == 1),
                    )
                lgo = lout.tile([1, R], dt.float32, tag="lgo")
                nc.scalar.activation(out=lgo[:], in_=lg[:], func=act_fn.Copy)
                nc.sync.dma_start(
                    lgs.ap().rearrange("(b j) -> b j", j=R)[S["b"] : S["b"] + 1, :],
                    lgo[:],
                )

        # tail: logits -> sigmoid -> scores (one ACT table switch total)
        jpp = (nblk * R) // P  # logits per partition
        lsb = fin.tile([P, jpp], dt.float32, tag="lsb")
        nc.sync.dma_start(lsb[:], lgs.ap().rearrange("(p j) -> p j", j=jpp))
        sig = fin.tile([P, jpp], dt.float32, tag="sig")
        nc.scalar.activation(out=sig[:], in_=lsb[:], func=act_fn.Sigmoid)
        nc.sync.dma_start(
            scores.ap().rearrange("(p j) o -> p (j o)", j=jpp), sig[:]
        )

    nc.compile()
    return nc


def _get_program(rows_per_core: int):
    if rows_per_core not in _cache:
        _cache[rows_per_core] = _build_program(rows_per_core)
    return _cache[rows_per_core]


def _build_runner(nc, n_cores):
    """Cached jitted PJRT runner (mirror of bass2jax.run_bass_via_pjrt without
    donation, so device args can be reused across calls)."""
    import jax
    from jax.sharding import Mesh, PartitionSpec
    from jax.experimental.shard_map import shard_map
    import concourse.mybir as mybir
    from concourse.bass2jax import (
        install_neuronx_cc_hook,
        partition_id_tensor,
        _bass_exec_p,
    )

    install_neuronx_cc_hook()
    partition_name = nc.partition_id_tensor.name if nc.partition_id_tensor else None

    in_names, out_names, out_avals, zero_outs = [], [], [], []
    for alloc in nc.m.functions[0].allocations:
        if not isinstance(alloc, mybir.MemoryLocationSet):
            continue
        name = alloc.memorylocations[0].name
        if alloc.kind == "ExternalInput":
            if name != partition_name:
                in_names.append(name)
        elif alloc.kind == "ExternalOutput":
            out_names.append(name)
            shape = tuple(alloc.tensor_shape)
            dtype = mybir.dt.np(alloc.dtype)
            out_avals.append(jax.core.ShapedArray(shape, dtype))
            zero_outs.append(np.zeros(shape, dtype))
    n_params = len(in_names)
    all_in_names = list(in_names) + list(out_names)
    if partition_name is not None:
        all_in_names.append(partition_name)

    def _body(*args):
        operands = list(args)
        if partition_name is not None:
            operands.append(partition_id_tensor())
        outs = _bass_exec_p.bind(
            *operands,
            out_avals=tuple(out_avals),
            in_names=tuple(all_in_names),
            out_names=tuple(out_names),
            lowering_input_output_aliases=(),
            sim_require_finite=True,
            sim_require_nnan=True,
            nc=nc,
        )
        return tuple(outs)

    devices = jax.devices()[:n_cores]
    assert len(devices) == n_cores
    mesh = Mesh(np.asarray(devices), ("core",))
    n_outs = len(out_names)
    fn = jax.jit(
        shard_map(
            _body, mesh=mesh,
            in_specs=(PartitionSpec("core"),) * (n_params + n_outs),
            out_specs=(PartitionSpec("core"),) * n_outs,
            check_rep=False,
        ),
        keep_unused=True,
    )
    return fn, mesh, in_names, out_names, zero_outs


def kernel(**inputs) -> np.ndarray:
    import jax
    from jax.sharding import NamedSharding, PartitionSpec

    arrs = _prep_host(inputs)
    candT = _prep_cand(inputs["candidate_embeddings"])  # [NCORES, P, nblk*4*R]
    n = np.asarray(inputs["candidate_embeddings"]).shape[0]
    rows_per_core = n // NCORES
    nc = _get_program(rows_per_core)

    if rows_per_core not in _runner_cache:
        _runner_cache[rows_per_core] = _build_runner(nc, NCORES)
    fn, mesh, in_names, out_names, zero_outs = _runner_cache[rows_per_core]

    per_core_named = {"candT": candT}  # [NCORES, ...] stacked
    concat_in = []
    for nm in in_names:
        if nm in per_core_named:
            a = per_core_named[nm].reshape(-1, *per_core_named[nm].shape[2:])
        else:
            a = np.concatenate([arrs[nm]] * NCORES, axis=0)
        concat_in.append(a)
    concat_zeros = [
        np.zeros((NCORES * z.shape[0], *z.shape[1:]), z.dtype) for z in zero_outs
    ]
    sh = NamedSharding(mesh, PartitionSpec("core"))
    dev_args = [jax.device_put(a, sh) for a in concat_in + concat_zeros]
    outs = fn(*dev_args)
    out_map = {nm: np.asarray(o) for nm, o in zip(out_names, outs)}
    return out_map["scores"].reshape(n, 1).astype(np.float32)


if __name__ == "__main__":
    rows = int(sys.argv[1]) if len(sys.argv) > 1 else 512
    nc = _build_program(rows)
    print("built ok:", rows)


# revision 75
# speedup vs baseline: 1.1520x; 1.1520x over previous
"""Trainium2 Bass kernel for nn_CrossAttentionReranker (feature-major rewrite).

Reference math (seq_len==1 everywhere) collapses:
  - softmax over a size-1 axis == 1, so MHA(x_q, x_kv) == x_kv @ wv.T @ out_w.T
    -> folded on host (fp64) into a single [512,512] matmul per layer.
  - ln_w == 1, ln_b == 0 and all biases == 0 in setup_inputs() (asserted),
    so LayerNorm is pure normalize.

Device dataflow (per core, data-parallel over candidate rows):
  FEATURE-MAJOR activations: [128 feature-partitions x 512 rows] x 4 chunks
  ([128, 2048] bf16 tiles).  Candidates are pre-transposed to feature-major
  on the HOST, so no PE transposes are needed on device.  Matmuls use
  resident weight chunks as lhsT and activations as rhs.  Residual adds:
  chunks 0-1 via identity-matmul in the same PSUM accumulation group,
  chunks 2-3 fused into the DVE PSUM-evacuation (tensor_tensor add).
  LN stats: a single (1/512)*ones[128x128] PE matmul per chunk group
  computes sum, /512 and partition-broadcast at once.  z^2 for the variance
  is computed on the otherwise-idle Pool (gpsimd) engine.  Normalize = two
  [128,2048] DVE ops using stride-0 broadcast APs on mu/rstd.  Blocks run
  in software-pipelined waves of 3 with LN ops interleaved op-type-major
  across the wave.  Sigmoid deferred to one tail pass via a DRAM logits
  scratch (single ACT table switch).
"""

import os
import sys

import numpy as np
import ml_dtypes

N = 131072
D = 512
HID = 256
L = 2
P = 128
R = 512          # rows per block
NCORES = 8
EPS = 1e-5
WAVE = int(os.environ.get("KERNEL_WAVE", "3"))

BF16 = ml_dtypes.bfloat16
F8 = ml_dtypes.float8_e4m3
FP8 = os.environ.get("KERNEL_FP8", "1") == "1"
WS = 32.0  # fp8 weight pre-scale (exact power of two; unfolded via evac scale)

_cache: dict = {}
_runner_cache: dict = {}


def _chunk(w: np.ndarray) -> np.ndarray:
    """[K, M] (K multiple of 128) -> [128, (K//128)*M], K-chunk-major on free dim."""
    k, m = w.shape
    assert k % P == 0
    return np.ascontiguousarray(
        w.reshape(k // P, P, m).transpose(1, 0, 2).reshape(P, (k // P) * m)
    )


def _prep_host(inputs):
    """Fold weights on host (fp64), cast to bf16, pre-chunk for lhsT layout."""
    f8 = np.float64
    assert np.all(np.asarray(inputs["ln_w"]) == 1.0), "kernel assumes ln_w == 1"
    assert not np.any(np.asarray(inputs["ln_b"])), "kernel assumes ln_b == 0"
    for k in ("attn_in_b", "attn_out_b", "ffn_b1", "ffn_b2", "head_b1", "head_b2"):
        assert not np.any(np.asarray(inputs[k])), f"kernel assumes {k} == 0"

    wdt = F8 if FP8 else BF16
    wsc = WS if FP8 else 1.0
    arrs = {}
    # wa and w2 feed LayerNorms whose residuals are themselves LN outputs
    # (feature-sum 0), so centering their OUTPUT dim on host makes every such
    # LN input exactly zero-mean: the device then skips mean subtraction
    # entirely (LN is invariant to the uniform shift removed here).
    for i in range(L):
        wv = np.asarray(inputs["attn_in_w"])[i][2 * D :].astype(f8)  # [D, D]
        ow = np.asarray(inputs["attn_out_w"])[i].astype(f8)          # [D, D]
        wa = wv.T @ ow.T                                             # x @ wa == mha(x)
        wa = wa - wa.mean(axis=1, keepdims=True)
        arrs[f"wa{i}"] = (_chunk(wa) * wsc).astype(wdt)              # [128, 4*512]
        w1 = np.asarray(inputs["ffn_w1"])[i].T.astype(f8)            # [512, 256]
        arrs[f"w1_{i}"] = (_chunk(w1) * wsc).astype(wdt)             # [128, 4*256]
        w2 = np.asarray(inputs["ffn_w2"])[i].T.astype(f8)            # [256, 512]
        w2 = w2 - w2.mean(axis=1, keepdims=True)
        arrs[f"w2_{i}"] = (_chunk(w2) * wsc).astype(wdt)             # [128, 2*512]
    arrs["h1"] = _chunk(np.asarray(inputs["head_w1"]).T.astype(f8)).astype(BF16)
    arrs["h2"] = _chunk(np.asarray(inputs["head_w2"]).T.astype(f8)).astype(BF16)
    # q0 appears only as an LN residual; centering it is exact (LN shift
    # invariance) and keeps that LN input zero-mean too.
    q0 = np.asarray(inputs["query_embedding"]).astype(f8).reshape(D)
    q0 = (q0 - q0.mean()).astype(np.float32)
    # feature-major q0, replicated along the row (free) dim: chunk j columns
    # all equal q0[j*128:(j+1)*128]
    q0T = np.concatenate(
        [np.broadcast_to(q0[j * P : (j + 1) * P][:, None], (P, R)) for j in range(4)],
        axis=1,
    )
    arrs["q0T"] = np.ascontiguousarray(q0T).astype(F8 if FP8 else BF16)  # [128, 4*512]
    # identity (scaled by WS under fp8 so PE residual adds match the psum scale)
    arrs["identb"] = (np.eye(P, dtype=np.float32) * wsc).astype(wdt)
    # all-(1/512) matrix: ones^T/D @ x = column means, broadcast to all partitions
    arrs["omat"] = np.full((P, P), 1.0 / D, dtype=np.float32).astype(BF16)
    return arrs


def _prep_cand(cand: np.ndarray) -> np.ndarray:
    """[N, D] row-major -> per-core feature-major chunked [NCORES, 128, nblk*4*R].

    candT[core, p, (b*4 + c)*R + j] == cand[core*rows + b*R + j, c*128 + p]
    """
    n = cand.shape[0]
    rows = n // NCORES
    nblk = rows // R
    x = np.asarray(cand).astype(F8 if FP8 else BF16).reshape(NCORES, nblk, R, 4, P)
    return np.ascontiguousarray(
        x.transpose(0, 4, 1, 3, 2).reshape(NCORES, P, nblk * 4 * R)
    )


def _build_program(rows_per_core: int):
    """Trace + schedule + compile the Bass program for one core (SPMD)."""
    import concourse.bass as bass
    import concourse.mybir as mybir
    import concourse.tile as tile
    from concourse import bacc
    from concourse.bass import ts

    dt = mybir.dt
    alu = mybir.AluOpType
    act_fn = mybir.ActivationFunctionType
    nblk = rows_per_core // R
    assert rows_per_core % R == 0

    nc = bacc.Bacc(
        "TRN2", target_bir_lowering=False, debug=False, num_devices=NCORES
    )

    wdt = dt.float8e4 if FP8 else dt.bfloat16
    adt = dt.float8e4 if FP8 else dt.bfloat16
    candT = nc.dram_tensor(
        "candT", [P, nblk * 4 * R], adt, kind="ExternalInput"
    )
    dr = {}
    for i in range(L):
        dr[f"wa{i}"] = nc.dram_tensor(f"wa{i}", [P, 4 * D], wdt, kind="ExternalInput")
        dr[f"w1_{i}"] = nc.dram_tensor(f"w1_{i}", [P, 4 * HID], wdt, kind="ExternalInput")
        dr[f"w2_{i}"] = nc.dram_tensor(f"w2_{i}", [P, 2 * D], wdt, kind="ExternalInput")
    dr["h1"] = nc.dram_tensor("h1", [P, 8 * HID], dt.bfloat16, kind="ExternalInput")
    dr["h2"] = nc.dram_tensor("h2", [P, 2], dt.bfloat16, kind="ExternalInput")
    dr["q0T"] = nc.dram_tensor("q0T", [P, 4 * R], adt, kind="ExternalInput")
    dr["identb"] = nc.dram_tensor("identb", [P, P], wdt, kind="ExternalInput")
    dr["omat"] = nc.dram_tensor("omat", [P, P], dt.bfloat16, kind="ExternalInput")
    lgs = nc.dram_tensor("lgs", [nblk * R], dt.float32, kind="Internal")
    scores = nc.dram_tensor("scores", [rows_per_core, 1], dt.float32, kind="ExternalOutput")

    from contextlib import ExitStack

    with tile.TileContext(nc) as tc, ExitStack() as ctx:
        const = ctx.enter_context(tc.tile_pool(name="const", bufs=1))

        def load_const(name, shape, dtype):
            t = const.tile(shape, dtype, tag=f"const_{name}")
            nc.sync.dma_start(t[:], dr[name].ap())
            return t

        wsb = []
        for i in range(L):
            wsb.append(
                (
                    load_const(f"wa{i}", [P, 4 * D], wdt),
                    load_const(f"w1_{i}", [P, 4 * HID], wdt),
                    load_const(f"w2_{i}", [P, 2 * D], wdt),
                )
            )
        h1sb = load_const("h1", [P, 8 * HID], dt.bfloat16)
        h2sb = load_const("h2", [P, 2], dt.bfloat16)
        q0T = load_const("q0T", [P, 4 * R], adt)
        identb = load_const("identb", [P, P], wdt)
        omat = load_const("omat", [P, P], dt.bfloat16)
        eps_t = const.tile([P, 1], dt.float32, tag="eps")
        nc.gpsimd.memset(eps_t[:], float(EPS))

        xp = ctx.enter_context(tc.tile_pool(name="xp", bufs=4))
        zp = ctx.enter_context(tc.tile_pool(name="zp", bufs=4))
        sqp = ctx.enter_context(tc.tile_pool(name="sqp", bufs=3))
        stp = ctx.enter_context(tc.tile_pool(name="stp", bufs=5))
        ap_ = ctx.enter_context(tc.tile_pool(name="ap", bufs=18))
        hp = ctx.enter_context(tc.tile_pool(name="hp", bufs=4))
        fin = ctx.enter_context(tc.tile_pool(name="fin", bufs=1))
        lout = ctx.enter_context(tc.tile_pool(name="lout", bufs=2))
        py = ctx.enter_context(tc.tile_pool(name="py", bufs=5, space="PSUM"))
        pst = ctx.enter_context(tc.tile_pool(name="pst", bufs=2, space="PSUM"))
        plg = ctx.enter_context(tc.tile_pool(name="plg", bufs=1, space="PSUM"))

        # residual chunks 0..PE_RES-1 added via identity matmul in PSUM;
        # chunks PE_RES..3 added by DVE during PSUM evacuation.
        PE_RES = int(os.environ.get("KERNEL_PE_RES", "2"))

        def pair2(ap_in, stride, num):
            """[128, X] AP -> [128, 2, num] AP pairing two adjacent k-tiles."""
            return bass.AP(
                ap_in.tensor, ap_in.offset, [ap_in.ap[0], [stride, 2], [1, num]]
            )

        def mm_stage(w_sb, rhs, nk, nfo, resid=None, force_bf16=False):
            """y^T chunks: out[fo] = sum_k w[k,fo-block]^T @ rhs[k] (+ resid[fo]).

            w_sb: [128, nk*nfo*128] chunked lhsT; rhs(k) -> [128, R] AP;
            resid(fo) -> [128, R] AP or None (PE-added only for fo < PE_RES).
            Under fp8, k-chunks are processed in pairs with DoubleRow perf
            mode (256-deep contraction per matmul at double rate).
            Returns list of PSUM tiles.
            """
            m = nfo * P
            ys = []
            for fo in range(nfo):
                pe_resid = resid is not None and fo < PE_RES
                y = py.tile([P, R], dt.float32, tag="y")
                if FP8 and not force_bf16:
                    assert nk % 2 == 0
                    for t in range(nk // 2):
                        w_ap = w_sb[:, 2 * t * m + fo * P : 2 * t * m + fo * P + P]
                        r_ap = rhs(2 * t)
                        nc.tensor.matmul(
                            y[:, :],
                            pair2(w_ap, m, P),
                            pair2(r_ap, R, R),
                            start=(t == 0),
                            stop=(t == nk // 2 - 1 and not pe_resid),
                            perf_mode=mybir.MatmulPerfMode.DoubleRow,
                        )
                else:
                    for k in range(nk):
                        nc.tensor.matmul(
                            y[:, :],
                            w_sb[:, k * m + fo * P : k * m + (fo + 1) * P],
                            rhs(k),
                            start=(k == 0),
                            stop=(k == nk - 1 and not pe_resid),
                        )
                if pe_resid:
                    nc.tensor.matmul(
                        y[:, :], identb[:], resid(fo), start=False, stop=True
                    )
                ys.append(y)
            return ys

        def ln_multi(ys_list, resid_list, out_dtype=None, twin=False,
                     need_mean=False):
            """LN for a wave of in-flight blocks; ops interleaved op-type-major
            across blocks so no engine stream has long dependent runs.

            Chunks >= PE_RES get their residual added during the DVE
            evacuation; chunks < PE_RES were already summed in PSUM.

            need_mean=False: the LN input is zero-mean by construction
            (host-centered weights + LN-output residuals), so the variance is
            just the second moment and no mean subtraction happens at all.
            need_mean=True (only layer-0 c-side, whose residual is the raw
            candidate embedding): full mean/variance path."""
            n = len(ys_list)
            Z, SQ, SBZ, SBQ, ME, MU2, VEPS, STD, RSTD, A = (
                [None] * n for _ in range(10)
            )
            for j, ys in enumerate(ys_list):
                _lvl = int(os.environ.get("KERNEL_LN_LEVEL", "3"))
                zpool = ap_ if _lvl < 3 else zp
                Z[j] = zpool.tile(
                    [P, 4 * R], adt if _lvl < 3 else dt.bfloat16,
                    name=f"z{j}", tag="z",
                )
                for c in range(4):
                    if c < PE_RES:
                        # psum holds WS*(W x + resid); evac unscales
                        nc.scalar.activation(
                            out=Z[j][:, ts(c, R)], in_=ys[c][:], func=act_fn.Copy,
                            scale=1.0 / WS if FP8 else 1.0,
                        )
                    elif FP8:
                        nc.vector.scalar_tensor_tensor(
                            out=Z[j][:, ts(c, R)], in0=ys[c][:],
                            scalar=1.0 / WS, in1=resid_list[j](c),
                            op0=alu.mult, op1=alu.add,
                        )
                    else:
                        nc.vector.tensor_tensor(
                            out=Z[j][:, ts(c, R)], in0=ys[c][:],
                            in1=resid_list[j](c), op=alu.add,
                        )
            LVL = int(os.environ.get("KERNEL_LN_LEVEL", "3"))
            NOSQ = os.environ.get("KERNEL_NOSQ", "0") == "1"
            NOSTATS = os.environ.get("KERNEL_NOSTATS", "0") == "1"
            NORECIP = os.environ.get("KERNEL_NORECIP", "0") == "1"
            NB = os.environ.get("KERNEL_NB", "0") == "1"
            if LVL == 0:
                return (Z, Z) if twin else Z
            # z^2 on DVE (bf16 SBUF-only tensor_tensor runs packed and is the
            # cheapest bulk elementwise path on HW; Act was the loaded engine
            # after the zero-mean restructure removed its other LN work).
            for j in range(n):
                SQ[j] = sqp.tile([P, 4 * R], dt.bfloat16, name=f"zsq{j}", tag="zsq")
                for h in range(2):
                    nc.vector.tensor_tensor(
                        out=SQ[j][:, h * 2 * R : (h + 1) * 2 * R],
                        in0=Z[j][:, h * 2 * R : (h + 1) * 2 * R],
                        in1=Z[j][:, h * 2 * R : (h + 1) * 2 * R],
                        op=alu.mult,
                    )
            # stats: sum + /512 + partition-broadcast in one PE op per chunk:
            # SBQ = (ones/512)^T @ sq-chunks, accumulated -> every partition
            # holds the column mean of z^2 (the variance, since z is
            # zero-mean unless need_mean).  The sum-z pass exists only for
            # the need_mean LN.
            if need_mean:
                for j in range(n):
                    SBZ[j] = py.tile([P, R], dt.float32, name=f"sbz{j}", tag="y")
                    for c in range(4):
                        nc.tensor.matmul(
                            SBZ[j][:], omat[:], Z[j][:, ts(c, R)],
                            start=(c == 0), stop=(c == 3),
                        )
            for j in range(n):
                SBQ[j] = pst.tile([P, R], dt.float32, name=f"sbq{j}", tag="sb")
                for c in range(4):
                    nc.tensor.matmul(
                        SBQ[j][:], omat[:], SQ[j][:, ts(c, R)],
                        start=(c == 0), stop=(c == 3),
                    )
            if need_mean:
                # ME and MU2 depend only on SBZ (done before the sumsq
                # matmuls), so both run during the sumsq pass.
                for j in range(n):
                    ME[j] = stp.tile([P, R], dt.bfloat16, name=f"me{j}", tag="me")
                    nc.scalar.activation(
                        out=ME[j][:], in_=SBZ[j][:], func=act_fn.Copy
                    )
            if LVL == 1:
                return (Z, Z) if twin else Z
            if need_mean:
                for j in range(n):
                    MU2[j] = stp.tile([P, R], dt.bfloat16, name=f"mu2{j}", tag="mu2")
                    nc.vector.tensor_tensor(
                        out=MU2[j][:], in0=ME[j][:], in1=ME[j][:], op=alu.mult
                    )
                for j in range(n):
                    VEPS[j] = stp.tile(
                        [P, R], dt.bfloat16, name=f"veps{j}", tag="veps"
                    )
                    nc.vector.scalar_tensor_tensor(
                        out=VEPS[j][:], in0=SBQ[j][:], scalar=1.0,
                        in1=MU2[j][:], op0=alu.bypass, op1=alu.subtract,
                    )
            for j in range(n):
                STD[j] = stp.tile([P, R], dt.float32, name=f"std{j}", tag="std")
                nc.scalar.activation(
                    out=STD[j][:],
                    in_=VEPS[j][:] if need_mean else SBQ[j][:],
                    func=act_fn.Sqrt, bias=eps_t[:],
                )

            if LVL == 2:
                return (Z, Z) if twin else Z

            for j in range(n):
                A[j] = ap_.tile(
                    [P, 4 * R], out_dtype or adt, name=f"a{j}", tag="a"
                )
            # normalize per chunk with plain 2-level stride-1 APs (stride-0
            # broadcast APs measured ~3x slower on HW); the [128,R] stat tile
            # already holds the per-column value on every partition.  Under
            # fp8 the subtract lands in the (now-free) bf16 SQ tile so the
            # activation is only fp8-quantized once, at the final multiply.
            if need_mean:
                AS = [SQ[j] if FP8 else A[j] for j in range(n)]
                for c in range(4):
                    for j in range(n):
                        nc.vector.tensor_tensor(
                            out=AS[j][:, ts(c, R)],
                            in0=Z[j][:, ts(c, R)],
                            in1=ME[j][:],
                            op=alu.subtract,
                        )
            else:
                AS = Z
            for j in range(n):
                # fp32 1/std via the fast custom-DVE op (InstReciprocal
                # measured ~4.5x slower on HW than the cost model claims),
                # then a bf16 convert on Act so the multiplies stay 2-byte.
                RSTDF = stp.tile([P, R], dt.float32, name=f"rstdf{j}", tag="rstdf")
                nc.vector.reciprocal_approx_fast(out=RSTDF[:], in_=STD[j][:])
                RSTD[j] = stp.tile([P, R], dt.bfloat16, name=f"rstd{j}", tag="rstd")
                nc.scalar.activation(
                    out=RSTD[j][:], in_=RSTDF[:], func=act_fn.Copy
                )
            for c in range(4):
                for j in range(n):
                    nc.vector.tensor_tensor(
                        out=A[j][:, ts(c, R)],
                        in0=AS[j][:, ts(c, R)],
                        in1=RSTD[j][:],
                        op=alu.mult,
                    )
            if not twin:
                return A
            # bf16 twin of the normalized output (for the bf16 head path)
            AB = [None] * n
            for j in range(n):
                AB[j] = ap_.tile(
                    [P, 4 * R], dt.bfloat16, name=f"ab{j}", tag="ab", bufs=WAVE + 1
                )
            for c in range(4):
                for j in range(n):
                    nc.vector.tensor_tensor(
                        out=AB[j][:, ts(c, R)],
                        in0=AS[j][:, ts(c, R)],
                        in1=RSTD[j][:],
                        op=alu.mult,
                    )
            return A, AB

        def input_stage(b):
            cT = xp.tile([P, 4 * R], adt, tag="cT")
            nc.sync.dma_start(cT[:], candT.ap()[:, b * 4 * R : (b + 1) * 4 * R])
            return cT

        def relu_multi(hps_list, out_dtype, tag, unscale=True):
            # relu is positively homogeneous: Relu(psum/WS) unscales exactly
            hs = []
            for j, hps in enumerate(hps_list):
                h = hp.tile([P, 2 * R], out_dtype, name=f"h{j}", tag=tag)
                for fo in range(2):
                    nc.scalar.activation(
                        out=h[:, ts(fo, R)], in_=hps[fo][:], func=act_fn.Relu,
                        scale=1.0 / WS if (FP8 and unscale) else 1.0,
                    )
                hs.append(h)
            return hs

        for w0 in range(0, nblk, WAVE):
            wb = list(range(w0, min(w0 + WAVE, nblk)))
            st = [{"b": b} for b in wb]
            for S in st:
                S["cT"] = input_stage(S["b"])
                S["q"], S["c"] = q0T, S["cT"]
            for i in range(L):
                wa, w1, w2 = wsb[i]
                for S in st:
                    S["y"] = mm_stage(
                        wa, lambda k, S=S: S["c"][:, ts(k, R)], 4, 4,
                        resid=lambda fo, S=S: S["q"][:, ts(fo, R)],
                    )
                a1s = ln_multi(
                    [S["y"] for S in st],
                    [lambda c, S=S: S["q"][:, ts(c, R)] for S in st],
                )
                for S, a1 in zip(st, a1s):
                    S["a1"] = a1
                    S["hps"] = mm_stage(w1, lambda k, a1=a1: a1[:, ts(k, R)], 4, 2)
                hs = relu_multi([S["hps"] for S in st], adt, "h")
                for S, h in zip(st, hs):
                    S["y"] = mm_stage(
                        w2, lambda k, h=h: h[:, ts(k, R)], 2, 4,
                        resid=lambda fo, S=S: S["a1"][:, ts(fo, R)],
                    )
                if FP8 and i == L - 1:
                    # bf16 twin of the final q for the bf16 head path
                    a2s, a2bs = ln_multi(
                        [S["y"] for S in st],
                        [lambda c, S=S: S["a1"][:, ts(c, R)] for S in st],
                        twin=True,
                    )
                else:
                    a2s = ln_multi(
                        [S["y"] for S in st],
                        [lambda c, S=S: S["a1"][:, ts(c, R)] for S in st],
                    )
                    a2bs = a2s
                for S, a2, a2b in zip(st, a2s, a2bs):
                    S["a2"] = a2
                    S["qh"] = a2b
                    S["y"] = mm_stage(
                        wa, lambda k, a2=a2: a2[:, ts(k, R)], 4, 4,
                        resid=lambda fo, S=S: S["c"][:, ts(fo, R)],
                    )
                a3s = ln_multi(
                    [S["y"] for S in st],
                    [lambda c, S=S: S["c"][:, ts(c, R)] for S in st],
                    need_mean=(i == 0),
                )
                for S, a3 in zip(st, a3s):
                    S["a3"] = a3
                    S["hps"] = mm_stage(w1, lambda k, a3=a3: a3[:, ts(k, R)], 4, 2)
                hs = relu_multi([S["hps"] for S in st], adt, "h")
                for S, h in zip(st, hs):
                    S["y"] = mm_stage(
                        w2, lambda k, h=h: h[:, ts(k, R)], 2, 4,
                        resid=lambda fo, S=S: S["a3"][:, ts(fo, R)],
                    )
                a4s = ln_multi(
                    [S["y"] for S in st],
                    [lambda c, S=S: S["a3"][:, ts(c, R)] for S in st],
                    out_dtype=dt.bfloat16 if i == L - 1 else None,
                )
                for S, a4 in zip(st, a4s):
                    S["q"], S["c"] = S["a2"], a4

            # head: combined = [q | c] -> HID -> 1, fully bf16 (head errors
            # reach the logits unwashed by any later LN)
            for S in st:
                S["hps"] = mm_stage(
                    h1sb,
                    lambda k, S=S: (
                        S["qh"][:, ts(k, R)] if k < 4 else S["c"][:, ts(k - 4, R)]
                    ),
                    8, 2, force_bf16=True,
                )
            hhs = relu_multi([S["hps"] for S in st], dt.bfloat16, "hh", unscale=False)
            for S, hh in zip(st, hhs):
                lg = plg.tile([1, R], dt.float32, tag="lg")
                for k in range(2):
                    nc.tensor.matmul(
                        lg[:, :], h2sb[:, k : k + 1], hh[:, ts(k, R)],
                        start=(k == 0), stop=(k == 1),
                    )
                lgo = lout.tile([1, R], dt.float32, tag="lgo")
                nc.scalar.activation(out=lgo[:], in_=lg[:], func=act_fn.Copy)
                nc.sync.dma_start(
                    lgs.ap().rearrange("(b j) -> b j", j=R)[S["b"] : S["b"] + 1, :],
                    lgo[:],
                )

        # tail: logits -> sigmoid -> scores (one ACT table switch total)
        jpp = (nblk * R) // P  # logits per partition
        lsb = fin.tile([P, jpp], dt.float32, tag="lsb")
        nc.sync.dma_start(lsb[:], lgs.ap().rearrange("(p j) -> p j", j=jpp))
        sig = fin.tile([P, jpp], dt.float32, tag="sig")
        nc.scalar.activation(out=sig[:], in_=lsb[:], func=act_fn.Sigmoid)
        nc.sync.dma_start(
            scores.ap().rearrange("(p j) o -> p (j o)", j=jpp), sig[:]
        )

    nc.compile()
    return nc


def _get_program(rows_per_core: int):
    if rows_per_core not in _cache:
        _cache[rows_per_core] = _build_program(rows_per_core)
    return _cache[rows_per_core]


def _build_runner(nc, n_cores):
    """Cached jitted PJRT runner (mirror of bass2jax.run_bass_via_pjrt without
    donation, so device args can be reused across calls)."""
    import jax
    from jax.sharding import Mesh, PartitionSpec
    from jax.experimental.shard_map import shard_map
    import concourse.mybir as mybir
    from concourse.bass2jax import (
        install_neuronx_cc_hook,
        partition_id_tensor,
        _bass_exec_p,
    )

    install_neuronx_cc_hook()
    partition_name = nc.partition_id_tensor.name if nc.partition_id_tensor else None

    in_names, out_names, out_avals, zero_outs = [], [], [], []
    for alloc in nc.m.functions[0].allocations:
        if not isinstance(alloc, mybir.MemoryLocationSet):
            continue
        name = alloc.memorylocations[0].name
        if alloc.kind == "ExternalInput":
            if name != partition_name:
                in_names.append(name)
        elif alloc.kind == "ExternalOutput":
            out_names.append(name)
            shape = tuple(alloc.tensor_shape)
            dtype = mybir.dt.np(alloc.dtype)
            out_avals.append(jax.core.ShapedArray(shape, dtype))
            zero_outs.append(np.zeros(shape, dtype))
    n_params = len(in_names)
    all_in_names = list(in_names) + list(out_names)
    if partition_name is not None:
        all_in_names.append(partition_name)

    def _body(*args):
        operands = list(args)
        if partition_name is not None:
            operands.append(partition_id_tensor())
        outs = _bass_exec_p.bind(
            *operands,
            out_avals=tuple(out_avals),
            in_names=tuple(all_in_names),
            out_names=tuple(out_names),
            lowering_input_output_aliases=(),
            sim_require_finite=True,
            sim_require_nnan=True,
            nc=nc,
        )
        return tuple(outs)

    devices = jax.devices()[:n_cores]
    assert len(devices) == n_cores
    mesh = Mesh(np.asarray(devices), ("core",))
    n_outs = len(out_names)
    fn = jax.jit(
        shard_map(
            _body, mesh=mesh,
            in_specs=(PartitionSpec("core"),) * (n_params + n_outs),
            out_specs=(PartitionSpec("core"),) * n_outs,
            check_rep=False,
        ),
        keep_unused=True,
    )
    return fn, mesh, in_names, out_names, zero_outs


def kernel(**inputs) -> np.ndarray:
    import jax
    from jax.sharding import NamedSharding, PartitionSpec

    arrs = _prep_host(inputs)
    candT = _prep_cand(inputs["candidate_embeddings"])  # [NCORES, P, nblk*4*R]
    n = np.asarray(inputs["candidate_embeddings"]).shape[0]
    rows_per_core = n // NCORES
    nc = _get_program(rows_per_core)

    if rows_per_core not in _runner_cache:
        _runner_cache[rows_per_core] = _build_runner(nc, NCORES)
    fn, mesh, in_names, out_names, zero_outs = _runner_cache[rows_per_core]

    per_core_named = {"candT": candT}  # [NCORES, ...] stacked
    concat_in = []
    for nm in in_names:
        if nm in per_core_named:
            a = per_core_named[nm].reshape(-1, *per_core_named[nm].shape[2:])
        else:
            a = np.concatenate([arrs[nm]] * NCORES, axis=0)
        concat_in.append(a)
    concat_zeros = [
        np.zeros((NCORES * z.shape[0], *z.shape[1:]), z.dtype) for z in zero_outs
    ]
    sh = NamedSharding(mesh, PartitionSpec("core"))
    dev_args = [jax.device_put(a, sh) for a in concat_in + concat_zeros]
    outs = fn(*dev_args)
    out_map = {nm: np.asarray(o) for nm, o in zip(out_names, outs)}
    return out_map["scores"].reshape(n, 1).astype(np.float32)


if __name__ == "__main__":
    rows = int(sys.argv[1]) if len(sys.argv) > 1 else 512
    nc = _build_program(rows)
    print("built ok:", rows)


# revision 76
# speedup vs baseline: 1.4245x; 1.2365x over previous
"""Trainium2 Bass kernel for nn_CrossAttentionReranker (feature-major rewrite).

Reference math (seq_len==1 everywhere) collapses:
  - softmax over a size-1 axis == 1, so MHA(x_q, x_kv) == x_kv @ wv.T @ out_w.T
    -> folded on host (fp64) into a single [512,512] matmul per layer.
  - ln_w == 1, ln_b == 0 and all biases == 0 in setup_inputs() (asserted),
    so LayerNorm is pure normalize.

Device dataflow (per core, data-parallel over candidate rows):
  FEATURE-MAJOR activations: [128 feature-partitions x 512 rows] x 4 chunks
  ([128, 2048] bf16 tiles).  Candidates are pre-transposed to feature-major
  on the HOST, so no PE transposes are needed on device.  Matmuls use
  resident weight chunks as lhsT and activations as rhs.  Residual adds:
  chunks 0-1 via identity-matmul in the same PSUM accumulation group,
  chunks 2-3 fused into the DVE PSUM-evacuation (tensor_tensor add).
  LN stats: a single (1/512)*ones[128x128] PE matmul per chunk group
  computes sum, /512 and partition-broadcast at once.  z^2 for the variance
  is computed on the otherwise-idle Pool (gpsimd) engine.  Normalize = two
  [128,2048] DVE ops using stride-0 broadcast APs on mu/rstd.  Blocks run
  in software-pipelined waves of 3 with LN ops interleaved op-type-major
  across the wave.  Sigmoid deferred to one tail pass via a DRAM logits
  scratch (single ACT table switch).
"""

import os
import sys

import numpy as np
import ml_dtypes

N = 131072
D = 512
HID = 256
L = 2
P = 128
R = 512          # rows per block
NCORES = 8
EPS = 1e-5
WAVE = int(os.environ.get("KERNEL_WAVE", "3"))

BF16 = ml_dtypes.bfloat16
F8 = ml_dtypes.float8_e4m3
FP8 = os.environ.get("KERNEL_FP8", "1") == "1"
WS = 32.0  # fp8 weight pre-scale (exact power of two; unfolded via evac scale)

_cache: dict = {}
_runner_cache: dict = {}


def _chunk(w: np.ndarray) -> np.ndarray:
    """[K, M] (K multiple of 128) -> [128, (K//128)*M], K-chunk-major on free dim."""
    k, m = w.shape
    assert k % P == 0
    return np.ascontiguousarray(
        w.reshape(k // P, P, m).transpose(1, 0, 2).reshape(P, (k // P) * m)
    )


def _prep_host(inputs):
    """Fold weights on host (fp64), cast to bf16, pre-chunk for lhsT layout."""
    f8 = np.float64
    assert np.all(np.asarray(inputs["ln_w"]) == 1.0), "kernel assumes ln_w == 1"
    assert not np.any(np.asarray(inputs["ln_b"])), "kernel assumes ln_b == 0"
    for k in ("attn_in_b", "attn_out_b", "ffn_b1", "ffn_b2", "head_b1", "head_b2"):
        assert not np.any(np.asarray(inputs[k])), f"kernel assumes {k} == 0"

    wdt = F8 if FP8 else BF16
    wsc = WS if FP8 else 1.0
    arrs = {}
    # wa and w2 feed LayerNorms whose residuals are themselves LN outputs
    # (feature-sum 0), so centering their OUTPUT dim on host makes every such
    # LN input exactly zero-mean: the device then skips mean subtraction
    # entirely (LN is invariant to the uniform shift removed here).
    for i in range(L):
        wv = np.asarray(inputs["attn_in_w"])[i][2 * D :].astype(f8)  # [D, D]
        ow = np.asarray(inputs["attn_out_w"])[i].astype(f8)          # [D, D]
        wa = wv.T @ ow.T                                             # x @ wa == mha(x)
        wa = wa - wa.mean(axis=1, keepdims=True)
        arrs[f"wa{i}"] = (_chunk(wa) * wsc).astype(wdt)              # [128, 4*512]
        w1 = np.asarray(inputs["ffn_w1"])[i].T.astype(f8)            # [512, 256]
        arrs[f"w1_{i}"] = (_chunk(w1) * wsc).astype(wdt)             # [128, 4*256]
        w2 = np.asarray(inputs["ffn_w2"])[i].T.astype(f8)            # [256, 512]
        w2 = w2 - w2.mean(axis=1, keepdims=True)
        arrs[f"w2_{i}"] = (_chunk(w2) * wsc).astype(wdt)             # [128, 2*512]
    arrs["h1"] = _chunk(np.asarray(inputs["head_w1"]).T.astype(f8)).astype(BF16)
    arrs["h2"] = _chunk(np.asarray(inputs["head_w2"]).T.astype(f8)).astype(BF16)
    # q0 appears only as an LN residual; centering it is exact (LN shift
    # invariance) and keeps that LN input zero-mean too.
    q0 = np.asarray(inputs["query_embedding"]).astype(f8).reshape(D)
    q0 = (q0 - q0.mean()).astype(np.float32)
    # feature-major q0, replicated along the row (free) dim: chunk j columns
    # all equal q0[j*128:(j+1)*128]
    q0T = np.concatenate(
        [np.broadcast_to(q0[j * P : (j + 1) * P][:, None], (P, R)) for j in range(4)],
        axis=1,
    )
    arrs["q0T"] = np.ascontiguousarray(q0T).astype(F8 if FP8 else BF16)  # [128, 4*512]
    # identity (scaled by WS under fp8 so PE residual adds match the psum scale)
    arrs["identb"] = (np.eye(P, dtype=np.float32) * wsc).astype(wdt)
    # all-(1/512) matrix: ones^T/D @ x = column means, broadcast to all partitions
    arrs["omat"] = np.full((P, P), 1.0 / D, dtype=np.float32).astype(BF16)
    return arrs


def _prep_cand(cand: np.ndarray) -> np.ndarray:
    """[N, D] row-major -> per-core feature-major chunked [NCORES, 128, nblk*4*R].

    candT[core, p, (b*4 + c)*R + j] == cand[core*rows + b*R + j, c*128 + p]
    """
    n = cand.shape[0]
    rows = n // NCORES
    nblk = rows // R
    x = np.asarray(cand).astype(F8 if FP8 else BF16).reshape(NCORES, nblk, R, 4, P)
    return np.ascontiguousarray(
        x.transpose(0, 4, 1, 3, 2).reshape(NCORES, P, nblk * 4 * R)
    )


def _build_program(rows_per_core: int):
    """Trace + schedule + compile the Bass program for one core (SPMD)."""
    import concourse.bass as bass
    import concourse.mybir as mybir
    import concourse.tile as tile
    from concourse import bacc
    from concourse.bass import ts

    dt = mybir.dt
    alu = mybir.AluOpType
    act_fn = mybir.ActivationFunctionType
    nblk = rows_per_core // R
    assert rows_per_core % R == 0

    nc = bacc.Bacc(
        "TRN2", target_bir_lowering=False, debug=False, num_devices=NCORES
    )

    wdt = dt.float8e4 if FP8 else dt.bfloat16
    adt = dt.float8e4 if FP8 else dt.bfloat16
    candT = nc.dram_tensor(
        "candT", [P, nblk * 4 * R], adt, kind="ExternalInput"
    )
    dr = {}
    for i in range(L):
        dr[f"wa{i}"] = nc.dram_tensor(f"wa{i}", [P, 4 * D], wdt, kind="ExternalInput")
        dr[f"w1_{i}"] = nc.dram_tensor(f"w1_{i}", [P, 4 * HID], wdt, kind="ExternalInput")
        dr[f"w2_{i}"] = nc.dram_tensor(f"w2_{i}", [P, 2 * D], wdt, kind="ExternalInput")
    dr["h1"] = nc.dram_tensor("h1", [P, 8 * HID], dt.bfloat16, kind="ExternalInput")
    dr["h2"] = nc.dram_tensor("h2", [P, 2], dt.bfloat16, kind="ExternalInput")
    dr["q0T"] = nc.dram_tensor("q0T", [P, 4 * R], adt, kind="ExternalInput")
    dr["identb"] = nc.dram_tensor("identb", [P, P], wdt, kind="ExternalInput")
    dr["omat"] = nc.dram_tensor("omat", [P, P], dt.bfloat16, kind="ExternalInput")
    lgs = nc.dram_tensor("lgs", [nblk * R], dt.float32, kind="Internal")
    scores = nc.dram_tensor("scores", [rows_per_core, 1], dt.float32, kind="ExternalOutput")

    from contextlib import ExitStack

    with tile.TileContext(nc) as tc, ExitStack() as ctx:
        const = ctx.enter_context(tc.tile_pool(name="const", bufs=1))

        def load_const(name, shape, dtype):
            t = const.tile(shape, dtype, tag=f"const_{name}")
            nc.sync.dma_start(t[:], dr[name].ap())
            return t

        wsb = []
        for i in range(L):
            wsb.append(
                (
                    load_const(f"wa{i}", [P, 4 * D], wdt),
                    load_const(f"w1_{i}", [P, 4 * HID], wdt),
                    load_const(f"w2_{i}", [P, 2 * D], wdt),
                )
            )
        h1sb = load_const("h1", [P, 8 * HID], dt.bfloat16)
        h2sb = load_const("h2", [P, 2], dt.bfloat16)
        q0T = load_const("q0T", [P, 4 * R], adt)
        identb = load_const("identb", [P, P], wdt)
        omat = load_const("omat", [P, P], dt.bfloat16)
        eps_t = const.tile([P, 1], dt.float32, tag="eps")
        nc.gpsimd.memset(eps_t[:], float(EPS))

        xp = ctx.enter_context(tc.tile_pool(name="xp", bufs=4))
        zp = ctx.enter_context(tc.tile_pool(name="zp", bufs=4))
        sqp = ctx.enter_context(tc.tile_pool(name="sqp", bufs=3))
        stp = ctx.enter_context(tc.tile_pool(name="stp", bufs=5))
        ap_ = ctx.enter_context(tc.tile_pool(name="ap", bufs=18))
        hp = ctx.enter_context(tc.tile_pool(name="hp", bufs=4))
        fin = ctx.enter_context(tc.tile_pool(name="fin", bufs=1))
        lout = ctx.enter_context(tc.tile_pool(name="lout", bufs=2))
        py = ctx.enter_context(tc.tile_pool(name="py", bufs=5, space="PSUM"))
        pst = ctx.enter_context(tc.tile_pool(name="pst", bufs=2, space="PSUM"))
        plg = ctx.enter_context(tc.tile_pool(name="plg", bufs=1, space="PSUM"))

        # residual chunks 0..PE_RES-1 added via identity matmul in PSUM;
        # chunks PE_RES..3 added by DVE during PSUM evacuation.
        PE_RES = int(os.environ.get("KERNEL_PE_RES", "2"))

        def pair2(ap_in, stride, num):
            """[128, X] AP -> [128, 2, num] AP pairing two adjacent k-tiles."""
            return bass.AP(
                ap_in.tensor, ap_in.offset, [ap_in.ap[0], [stride, 2], [1, num]]
            )

        def mm_stage(w_sb, rhs, nk, nfo, resid=None, force_bf16=False):
            """y^T chunks: out[fo] = sum_k w[k,fo-block]^T @ rhs[k] (+ resid[fo]).

            w_sb: [128, nk*nfo*128] chunked lhsT; rhs(k) -> [128, R] AP;
            resid(fo) -> [128, R] AP or None (PE-added only for fo < PE_RES).
            Under fp8, k-chunks are processed in pairs with DoubleRow perf
            mode (256-deep contraction per matmul at double rate).
            Returns list of PSUM tiles.
            """
            m = nfo * P
            ys = []
            for fo in range(nfo):
                pe_resid = resid is not None and fo < PE_RES
                y = py.tile([P, R], dt.float32, tag="y")
                if FP8 and not force_bf16:
                    assert nk % 2 == 0
                    for t in range(nk // 2):
                        w_ap = w_sb[:, 2 * t * m + fo * P : 2 * t * m + fo * P + P]
                        r_ap = rhs(2 * t)
                        nc.tensor.matmul(
                            y[:, :],
                            pair2(w_ap, m, P),
                            pair2(r_ap, R, R),
                            start=(t == 0),
                            stop=(t == nk // 2 - 1 and not pe_resid),
                            perf_mode=mybir.MatmulPerfMode.DoubleRow,
                        )
                else:
                    for k in range(nk):
                        nc.tensor.matmul(
                            y[:, :],
                            w_sb[:, k * m + fo * P : k * m + (fo + 1) * P],
                            rhs(k),
                            start=(k == 0),
                            stop=(k == nk - 1 and not pe_resid),
                        )
                if pe_resid:
                    nc.tensor.matmul(
                        y[:, :], identb[:], resid(fo), start=False, stop=True
                    )
                ys.append(y)
            return ys

        def ln_multi(ys_list, resid_list, out_dtype=None, twin=False,
                     need_mean=False):
            """LN for a wave of in-flight blocks; ops interleaved op-type-major
            across blocks so no engine stream has long dependent runs.

            Chunks >= PE_RES get their residual added during the DVE
            evacuation; chunks < PE_RES were already summed in PSUM.

            need_mean=False: the LN input is zero-mean by construction
            (host-centered weights + LN-output residuals), so the variance is
            just the second moment and no mean subtraction happens at all.
            need_mean=True (only layer-0 c-side, whose residual is the raw
            candidate embedding): full mean/variance path."""
            n = len(ys_list)
            Z, SQ, SBZ, SBQ, ME, MU2, VEPS, STD, RSTD, A = (
                [None] * n for _ in range(10)
            )
            for j, ys in enumerate(ys_list):
                _lvl = int(os.environ.get("KERNEL_LN_LEVEL", "3"))
                zpool = ap_ if _lvl < 3 else zp
                Z[j] = zpool.tile(
                    [P, 4 * R], adt if _lvl < 3 else dt.bfloat16,
                    name=f"z{j}", tag="z",
                )
                for c in range(4):
                    if c < PE_RES:
                        # psum holds WS*(W x + resid); evac unscales
                        nc.scalar.activation(
                            out=Z[j][:, ts(c, R)], in_=ys[c][:], func=act_fn.Copy,
                            scale=1.0 / WS if FP8 else 1.0,
                        )
                    elif FP8:
                        nc.vector.scalar_tensor_tensor(
                            out=Z[j][:, ts(c, R)], in0=ys[c][:],
                            scalar=1.0 / WS, in1=resid_list[j](c),
                            op0=alu.mult, op1=alu.add,
                        )
                    else:
                        nc.vector.tensor_tensor(
                            out=Z[j][:, ts(c, R)], in0=ys[c][:],
                            in1=resid_list[j](c), op=alu.add,
                        )
            LVL = int(os.environ.get("KERNEL_LN_LEVEL", "3"))
            NOSQ = os.environ.get("KERNEL_NOSQ", "0") == "1"
            NOSTATS = os.environ.get("KERNEL_NOSTATS", "0") == "1"
            NORECIP = os.environ.get("KERNEL_NORECIP", "0") == "1"
            NB = os.environ.get("KERNEL_NB", "0") == "1"
            if LVL == 0:
                return (Z, Z) if twin else Z
            # z^2 on DVE (bf16 SBUF-only tensor_tensor runs packed and is the
            # cheapest bulk elementwise path on HW; Act was the loaded engine
            # after the zero-mean restructure removed its other LN work).
            SQ_SPLIT = os.environ.get("KERNEL_SQ_SPLIT", "0") == "1"
            for j in range(n):
                SQ[j] = sqp.tile([P, 4 * R], dt.bfloat16, name=f"zsq{j}", tag="zsq")
                if SQ_SPLIT:
                    nc.scalar.activation(
                        out=SQ[j][:, 0 : 2 * R], in_=Z[j][:, 0 : 2 * R],
                        func=act_fn.Square,
                    )
                    nc.vector.tensor_tensor(
                        out=SQ[j][:, 2 * R : 4 * R], in0=Z[j][:, 2 * R : 4 * R],
                        in1=Z[j][:, 2 * R : 4 * R], op=alu.mult,
                    )
                else:
                    for h in range(2):
                        nc.vector.tensor_tensor(
                            out=SQ[j][:, h * 2 * R : (h + 1) * 2 * R],
                            in0=Z[j][:, h * 2 * R : (h + 1) * 2 * R],
                            in1=Z[j][:, h * 2 * R : (h + 1) * 2 * R],
                            op=alu.mult,
                        )
            # stats: sum + /512 + partition-broadcast in one PE op per chunk:
            # SBQ = (ones/512)^T @ sq-chunks, accumulated -> every partition
            # holds the column mean of z^2 (the variance, since z is
            # zero-mean unless need_mean).  The sum-z pass exists only for
            # the need_mean LN.
            if need_mean:
                for j in range(n):
                    SBZ[j] = py.tile([P, R], dt.float32, name=f"sbz{j}", tag="y")
                    for c in range(4):
                        nc.tensor.matmul(
                            SBZ[j][:], omat[:], Z[j][:, ts(c, R)],
                            start=(c == 0), stop=(c == 3),
                        )
            for j in range(n):
                SBQ[j] = pst.tile([P, R], dt.float32, name=f"sbq{j}", tag="sb")
                for c in range(4):
                    nc.tensor.matmul(
                        SBQ[j][:], omat[:], SQ[j][:, ts(c, R)],
                        start=(c == 0), stop=(c == 3),
                    )
            if need_mean:
                # ME and MU2 depend only on SBZ (done before the sumsq
                # matmuls), so both run during the sumsq pass.
                for j in range(n):
                    ME[j] = stp.tile([P, R], dt.bfloat16, name=f"me{j}", tag="me")
                    nc.scalar.activation(
                        out=ME[j][:], in_=SBZ[j][:], func=act_fn.Copy
                    )
            if LVL == 1:
                return (Z, Z) if twin else Z
            if need_mean:
                for j in range(n):
                    MU2[j] = stp.tile([P, R], dt.bfloat16, name=f"mu2{j}", tag="mu2")
                    nc.vector.tensor_tensor(
                        out=MU2[j][:], in0=ME[j][:], in1=ME[j][:], op=alu.mult
                    )
                for j in range(n):
                    VEPS[j] = stp.tile(
                        [P, R], dt.bfloat16, name=f"veps{j}", tag="veps"
                    )
                    nc.vector.scalar_tensor_tensor(
                        out=VEPS[j][:], in0=SBQ[j][:], scalar=1.0,
                        in1=MU2[j][:], op0=alu.bypass, op1=alu.subtract,
                    )
            for j in range(n):
                STD[j] = stp.tile([P, R], dt.float32, name=f"std{j}", tag="std")
                nc.scalar.activation(
                    out=STD[j][:],
                    in_=VEPS[j][:] if need_mean else SBQ[j][:],
                    func=act_fn.Sqrt, bias=eps_t[:],
                )

            if LVL == 2:
                return (Z, Z) if twin else Z

            for j in range(n):
                A[j] = ap_.tile(
                    [P, 4 * R], out_dtype or adt, name=f"a{j}", tag="a"
                )
            # normalize per chunk with plain 2-level stride-1 APs (stride-0
            # broadcast APs measured ~3x slower on HW); the [128,R] stat tile
            # already holds the per-column value on every partition.  Under
            # fp8 the subtract lands in the (now-free) bf16 SQ tile so the
            # activation is only fp8-quantized once, at the final multiply.
            if need_mean:
                AS = [SQ[j] if FP8 else A[j] for j in range(n)]
                for c in range(4):
                    for j in range(n):
                        nc.vector.tensor_tensor(
                            out=AS[j][:, ts(c, R)],
                            in0=Z[j][:, ts(c, R)],
                            in1=ME[j][:],
                            op=alu.subtract,
                        )
            else:
                AS = Z
            for j in range(n):
                # fp32 1/std via the fast custom-DVE op (InstReciprocal
                # measured ~4.5x slower on HW than the cost model claims),
                # then a bf16 convert on Act so the multiplies stay 2-byte.
                RSTDF = stp.tile([P, R], dt.float32, name=f"rstdf{j}", tag="rstdf")
                nc.vector.reciprocal_approx_fast(out=RSTDF[:], in_=STD[j][:])
                RSTD[j] = stp.tile([P, R], dt.bfloat16, name=f"rstd{j}", tag="rstd")
                nc.scalar.activation(
                    out=RSTD[j][:], in_=RSTDF[:], func=act_fn.Copy
                )
            for c in range(4):
                for j in range(n):
                    nc.vector.tensor_tensor(
                        out=A[j][:, ts(c, R)],
                        in0=AS[j][:, ts(c, R)],
                        in1=RSTD[j][:],
                        op=alu.mult,
                    )
            if not twin:
                return A
            # bf16 twin of the normalized output (for the bf16 head path)
            AB = [None] * n
            for j in range(n):
                AB[j] = ap_.tile(
                    [P, 4 * R], dt.bfloat16, name=f"ab{j}", tag="ab", bufs=WAVE + 1
                )
            for c in range(4):
                for j in range(n):
                    nc.vector.tensor_tensor(
                        out=AB[j][:, ts(c, R)],
                        in0=AS[j][:, ts(c, R)],
                        in1=RSTD[j][:],
                        op=alu.mult,
                    )
            return A, AB

        def input_stage(b):
            cT = xp.tile([P, 4 * R], adt, tag="cT")
            nc.sync.dma_start(cT[:], candT.ap()[:, b * 4 * R : (b + 1) * 4 * R])
            return cT

        def relu_multi(hps_list, out_dtype, tag, unscale=True):
            # relu is positively homogeneous: Relu(psum/WS) unscales exactly
            hs = []
            for j, hps in enumerate(hps_list):
                h = hp.tile([P, 2 * R], out_dtype, name=f"h{j}", tag=tag)
                for fo in range(2):
                    nc.scalar.activation(
                        out=h[:, ts(fo, R)], in_=hps[fo][:], func=act_fn.Relu,
                        scale=1.0 / WS if (FP8 and unscale) else 1.0,
                    )
                hs.append(h)
            return hs

        for w0 in range(0, nblk, WAVE):
            wb = list(range(w0, min(w0 + WAVE, nblk)))
            st = [{"b": b} for b in wb]
            for S in st:
                S["cT"] = input_stage(S["b"])
                S["q"], S["c"] = q0T, S["cT"]
            for i in range(L):
                wa, w1, w2 = wsb[i]
                for S in st:
                    S["y"] = mm_stage(
                        wa, lambda k, S=S: S["c"][:, ts(k, R)], 4, 4,
                        resid=lambda fo, S=S: S["q"][:, ts(fo, R)],
                    )
                a1s = ln_multi(
                    [S["y"] for S in st],
                    [lambda c, S=S: S["q"][:, ts(c, R)] for S in st],
                )
                for S, a1 in zip(st, a1s):
                    S["a1"] = a1
                    S["hps"] = mm_stage(w1, lambda k, a1=a1: a1[:, ts(k, R)], 4, 2)
                hs = relu_multi([S["hps"] for S in st], adt, "h")
                for S, h in zip(st, hs):
                    S["y"] = mm_stage(
                        w2, lambda k, h=h: h[:, ts(k, R)], 2, 4,
                        resid=lambda fo, S=S: S["a1"][:, ts(fo, R)],
                    )
                if FP8 and i == L - 1:
                    # bf16 twin of the final q for the bf16 head path
                    a2s, a2bs = ln_multi(
                        [S["y"] for S in st],
                        [lambda c, S=S: S["a1"][:, ts(c, R)] for S in st],
                        twin=True,
                    )
                else:
                    a2s = ln_multi(
                        [S["y"] for S in st],
                        [lambda c, S=S: S["a1"][:, ts(c, R)] for S in st],
                    )
                    a2bs = a2s
                for S, a2, a2b in zip(st, a2s, a2bs):
                    S["a2"] = a2
                    S["qh"] = a2b
                    S["y"] = mm_stage(
                        wa, lambda k, a2=a2: a2[:, ts(k, R)], 4, 4,
                        resid=lambda fo, S=S: S["c"][:, ts(fo, R)],
                    )
                a3s = ln_multi(
                    [S["y"] for S in st],
                    [lambda c, S=S: S["c"][:, ts(c, R)] for S in st],
                    need_mean=(i == 0),
                )
                for S, a3 in zip(st, a3s):
                    S["a3"] = a3
                    S["hps"] = mm_stage(w1, lambda k, a3=a3: a3[:, ts(k, R)], 4, 2)
                hs = relu_multi([S["hps"] for S in st], adt, "h")
                for S, h in zip(st, hs):
                    S["y"] = mm_stage(
                        w2, lambda k, h=h: h[:, ts(k, R)], 2, 4,
                        resid=lambda fo, S=S: S["a3"][:, ts(fo, R)],
                    )
                a4s = ln_multi(
                    [S["y"] for S in st],
                    [lambda c, S=S: S["a3"][:, ts(c, R)] for S in st],
                    out_dtype=dt.bfloat16 if i == L - 1 else None,
                )
                for S, a4 in zip(st, a4s):
                    S["q"], S["c"] = S["a2"], a4

            # head: combined = [q | c] -> HID -> 1, fully bf16 (head errors
            # reach the logits unwashed by any later LN)
            for S in st:
                S["hps"] = mm_stage(
                    h1sb,
                    lambda k, S=S: (
                        S["qh"][:, ts(k, R)] if k < 4 else S["c"][:, ts(k - 4, R)]
                    ),
                    8, 2, force_bf16=True,
                )
            hhs = relu_multi([S["hps"] for S in st], dt.bfloat16, "hh", unscale=False)
            for S, hh in zip(st, hhs):
                lg = plg.tile([1, R], dt.float32, tag="lg")
                for k in range(2):
                    nc.tensor.matmul(
                        lg[:, :], h2sb[:, k : k + 1], hh[:, ts(k, R)],
                        start=(k == 0), stop=(k == 1),
                    )
                lgo = lout.tile([1, R], dt.float32, tag="lgo")
                nc.scalar.activation(out=lgo[:], in_=lg[:], func=act_fn.Copy)
                nc.sync.dma_start(
                    lgs.ap().rearrange("(b j) -> b j", j=R)[S["b"] : S["b"] + 1, :],
                    lgo[:],
                )

        # tail: logits -> sigmoid -> scores (one ACT table switch total)
        jpp = (nblk * R) // P  # logits per partition
        lsb = fin.tile([P, jpp], dt.float32, tag="lsb")
        nc.sync.dma_start(lsb[:], lgs.ap().rearrange("(p j) -> p j", j=jpp))
        sig = fin.tile([P, jpp], dt.float32, tag="sig")
        nc.scalar.activation(out=sig[:], in_=lsb[:], func=act_fn.Sigmoid)
        nc.sync.dma_start(
            scores.ap().rearrange("(p j) o -> p (j o)", j=jpp), sig[:]
        )

    nc.compile()
    return nc


def _get_program(rows_per_core: int):
    if rows_per_core not in _cache:
        _cache[rows_per_core] = _build_program(rows_per_core)
    return _cache[rows_per_core]


def _build_runner(nc, n_cores):
    """Cached jitted PJRT runner (mirror of bass2jax.run_bass_via_pjrt without
    donation, so device args can be reused across calls)."""
    import jax
    from jax.sharding import Mesh, PartitionSpec
    from jax.experimental.shard_map import shard_map
    import concourse.mybir as mybir
    from concourse.bass2jax import (
        install_neuronx_cc_hook,
        partition_id_tensor,
        _bass_exec_p,
    )

    install_neuronx_cc_hook()
    partition_name = nc.partition_id_tensor.name if nc.partition_id_tensor else None

    in_names, out_names, out_avals, zero_outs = [], [], [], []
    for alloc in nc.m.functions[0].allocations:
        if not isinstance(alloc, mybir.MemoryLocationSet):
            continue
        name = alloc.memorylocations[0].name
        if alloc.kind == "ExternalInput":
            if name != partition_name:
                in_names.append(name)
        elif alloc.kind == "ExternalOutput":
            out_names.append(name)
            shape = tuple(alloc.tensor_shape)
            dtype = mybir.dt.np(alloc.dtype)
            out_avals.append(jax.core.ShapedArray(shape, dtype))
            zero_outs.append(np.zeros(shape, dtype))
    n_params = len(in_names)
    all_in_names = list(in_names) + list(out_names)
    if partition_name is not None:
        all_in_names.append(partition_name)

    def _body(*args):
        operands = list(args)
        if partition_name is not None:
            operands.append(partition_id_tensor())
        outs = _bass_exec_p.bind(
            *operands,
            out_avals=tuple(out_avals),
            in_names=tuple(all_in_names),
            out_names=tuple(out_names),
            lowering_input_output_aliases=(),
            sim_require_finite=True,
            sim_require_nnan=True,
            nc=nc,
        )
        return tuple(outs)

    devices = jax.devices()[:n_cores]
    assert len(devices) == n_cores
    mesh = Mesh(np.asarray(devices), ("core",))
    n_outs = len(out_names)
    fn = jax.jit(
        shard_map(
            _body, mesh=mesh,
            in_specs=(PartitionSpec("core"),) * (n_params + n_outs),
            out_specs=(PartitionSpec("core"),) * n_outs,
            check_rep=False,
        ),
        keep_unused=True,
    )
    return fn, mesh, in_names, out_names, zero_outs


def kernel(**inputs) -> np.ndarray:
    import jax
    from jax.sharding import NamedSharding, PartitionSpec

    arrs = _prep_host(inputs)
    candT = _prep_cand(inputs["candidate_embeddings"])  # [NCORES, P, nblk*4*R]
    n = np.asarray(inputs["candidate_embeddings"]).shape[0]
    rows_per_core = n // NCORES
    nc = _get_program(rows_per_core)

    if rows_per_core not in _runner_cache:
        _runner_cache[rows_per_core] = _build_runner(nc, NCORES)
    fn, mesh, in_names, out_names, zero_outs = _runner_cache[rows_per_core]

    per_core_named = {"candT": candT}  # [NCORES, ...] stacked
    concat_in = []
    for nm in in_names:
        if nm in per_core_named:
            a = per_core_named[nm].reshape(-1, *per_core_named[nm].shape[2:])
        else:
            a = np.concatenate([arrs[nm]] * NCORES, axis=0)
        concat_in.append(a)
    concat_zeros = [
        np.zeros((NCORES * z.shape[0], *z.shape[1:]), z.dtype) for z in zero_outs
    ]
    sh = NamedSharding(mesh, PartitionSpec("core"))
    dev_args = [jax.device_put(a, sh) for a in concat_in + concat_zeros]
    outs = fn(*dev_args)
    out_map = {nm: np.asarray(o) for nm, o in zip(out_names, outs)}
    return out_map["scores"].reshape(n, 1).astype(np.float32)


if __name__ == "__main__":
    rows = int(sys.argv[1]) if len(sys.argv) > 1 else 512
    nc = _build_program(rows)
    print("built ok:", rows)


# revision 77
# speedup vs baseline: 1.4530x; 1.0200x over previous
"""Trainium2 Bass kernel for nn_CrossAttentionReranker (feature-major rewrite).

Reference math (seq_len==1 everywhere) collapses:
  - softmax over a size-1 axis == 1, so MHA(x_q, x_kv) == x_kv @ wv.T @ out_w.T
    -> folded on host (fp64) into a single [512,512] matmul per layer.
  - ln_w == 1, ln_b == 0 and all biases == 0 in setup_inputs() (asserted),
    so LayerNorm is pure normalize.

Device dataflow (per core, data-parallel over candidate rows):
  FEATURE-MAJOR activations: [128 feature-partitions x 512 rows] x 4 chunks
  ([128, 2048] bf16 tiles).  Candidates are pre-transposed to feature-major
  on the HOST, so no PE transposes are needed on device.  Matmuls use
  resident weight chunks as lhsT and activations as rhs.  Residual adds:
  chunks 0-1 via identity-matmul in the same PSUM accumulation group,
  chunks 2-3 fused into the DVE PSUM-evacuation (tensor_tensor add).
  LN stats: a single (1/512)*ones[128x128] PE matmul per chunk group
  computes sum, /512 and partition-broadcast at once.  z^2 for the variance
  is computed on the otherwise-idle Pool (gpsimd) engine.  Normalize = two
  [128,2048] DVE ops using stride-0 broadcast APs on mu/rstd.  Blocks run
  in software-pipelined waves of 3 with LN ops interleaved op-type-major
  across the wave.  Sigmoid deferred to one tail pass via a DRAM logits
  scratch (single ACT table switch).
"""

import os
import sys

import numpy as np
import ml_dtypes

N = 131072
D = 512
HID = 256
L = 2
P = 128
R = 512          # rows per block
NCORES = 8
EPS = 1e-5
WAVE = int(os.environ.get("KERNEL_WAVE", "3"))

BF16 = ml_dtypes.bfloat16
F8 = ml_dtypes.float8_e4m3
FP8 = os.environ.get("KERNEL_FP8", "1") == "1"
WS = 32.0  # fp8 weight pre-scale (exact power of two; unfolded via evac scale)

_cache: dict = {}
_runner_cache: dict = {}


def _chunk(w: np.ndarray) -> np.ndarray:
    """[K, M] (K multiple of 128) -> [128, (K//128)*M], K-chunk-major on free dim."""
    k, m = w.shape
    assert k % P == 0
    return np.ascontiguousarray(
        w.reshape(k // P, P, m).transpose(1, 0, 2).reshape(P, (k // P) * m)
    )


def _prep_host(inputs):
    """Fold weights on host (fp64), cast to bf16, pre-chunk for lhsT layout."""
    f8 = np.float64
    assert np.all(np.asarray(inputs["ln_w"]) == 1.0), "kernel assumes ln_w == 1"
    assert not np.any(np.asarray(inputs["ln_b"])), "kernel assumes ln_b == 0"
    for k in ("attn_in_b", "attn_out_b", "ffn_b1", "ffn_b2", "head_b1", "head_b2"):
        assert not np.any(np.asarray(inputs[k])), f"kernel assumes {k} == 0"

    wdt = F8 if FP8 else BF16
    wsc = WS if FP8 else 1.0
    arrs = {}
    # wa and w2 feed LayerNorms whose residuals are themselves LN outputs
    # (feature-sum 0), so centering their OUTPUT dim on host makes every such
    # LN input exactly zero-mean: the device then skips mean subtraction
    # entirely (LN is invariant to the uniform shift removed here).
    for i in range(L):
        wv = np.asarray(inputs["attn_in_w"])[i][2 * D :].astype(f8)  # [D, D]
        ow = np.asarray(inputs["attn_out_w"])[i].astype(f8)          # [D, D]
        wa = wv.T @ ow.T                                             # x @ wa == mha(x)
        wa = wa - wa.mean(axis=1, keepdims=True)
        arrs[f"wa{i}"] = (_chunk(wa) * wsc).astype(wdt)              # [128, 4*512]
        w1 = np.asarray(inputs["ffn_w1"])[i].T.astype(f8)            # [512, 256]
        arrs[f"w1_{i}"] = (_chunk(w1) * wsc).astype(wdt)             # [128, 4*256]
        w2 = np.asarray(inputs["ffn_w2"])[i].T.astype(f8)            # [256, 512]
        w2 = w2 - w2.mean(axis=1, keepdims=True)
        arrs[f"w2_{i}"] = (_chunk(w2) * wsc).astype(wdt)             # [128, 2*512]
    arrs["h1"] = _chunk(np.asarray(inputs["head_w1"]).T.astype(f8)).astype(BF16)
    arrs["h2"] = _chunk(np.asarray(inputs["head_w2"]).T.astype(f8)).astype(BF16)
    # q0 appears only as an LN residual; centering it is exact (LN shift
    # invariance) and keeps that LN input zero-mean too.
    q0 = np.asarray(inputs["query_embedding"]).astype(f8).reshape(D)
    q0 = (q0 - q0.mean()).astype(np.float32)
    # feature-major q0, replicated along the row (free) dim: chunk j columns
    # all equal q0[j*128:(j+1)*128]
    q0T = np.concatenate(
        [np.broadcast_to(q0[j * P : (j + 1) * P][:, None], (P, R)) for j in range(4)],
        axis=1,
    )
    arrs["q0T"] = np.ascontiguousarray(q0T).astype(F8 if FP8 else BF16)  # [128, 4*512]
    # identity (scaled by WS under fp8 so PE residual adds match the psum scale)
    arrs["identb"] = (np.eye(P, dtype=np.float32) * wsc).astype(wdt)
    # all-(1/512) matrix: ones^T/D @ x = column means, broadcast to all partitions
    arrs["omat"] = np.full((P, P), 1.0 / D, dtype=np.float32).astype(BF16)
    return arrs


def _prep_cand(cand: np.ndarray) -> np.ndarray:
    """[N, D] row-major -> per-core feature-major chunked [NCORES, 128, nblk*4*R].

    candT[core, p, (b*4 + c)*R + j] == cand[core*rows + b*R + j, c*128 + p]
    """
    n = cand.shape[0]
    rows = n // NCORES
    nblk = rows // R
    x = np.asarray(cand).astype(F8 if FP8 else BF16).reshape(NCORES, nblk, R, 4, P)
    return np.ascontiguousarray(
        x.transpose(0, 4, 1, 3, 2).reshape(NCORES, P, nblk * 4 * R)
    )


def _build_program(rows_per_core: int):
    """Trace + schedule + compile the Bass program for one core (SPMD)."""
    import concourse.bass as bass
    import concourse.mybir as mybir
    import concourse.tile as tile
    from concourse import bacc
    from concourse.bass import ts

    dt = mybir.dt
    alu = mybir.AluOpType
    act_fn = mybir.ActivationFunctionType
    nblk = rows_per_core // R
    assert rows_per_core % R == 0

    nc = bacc.Bacc(
        "TRN2", target_bir_lowering=False, debug=False, num_devices=NCORES
    )

    wdt = dt.float8e4 if FP8 else dt.bfloat16
    adt = dt.float8e4 if FP8 else dt.bfloat16
    candT = nc.dram_tensor(
        "candT", [P, nblk * 4 * R], adt, kind="ExternalInput"
    )
    dr = {}
    for i in range(L):
        dr[f"wa{i}"] = nc.dram_tensor(f"wa{i}", [P, 4 * D], wdt, kind="ExternalInput")
        dr[f"w1_{i}"] = nc.dram_tensor(f"w1_{i}", [P, 4 * HID], wdt, kind="ExternalInput")
        dr[f"w2_{i}"] = nc.dram_tensor(f"w2_{i}", [P, 2 * D], wdt, kind="ExternalInput")
    dr["h1"] = nc.dram_tensor("h1", [P, 8 * HID], dt.bfloat16, kind="ExternalInput")
    dr["h2"] = nc.dram_tensor("h2", [P, 2], dt.bfloat16, kind="ExternalInput")
    dr["q0T"] = nc.dram_tensor("q0T", [P, 4 * R], adt, kind="ExternalInput")
    dr["identb"] = nc.dram_tensor("identb", [P, P], wdt, kind="ExternalInput")
    dr["omat"] = nc.dram_tensor("omat", [P, P], dt.bfloat16, kind="ExternalInput")
    lgs = nc.dram_tensor("lgs", [nblk * R], dt.float32, kind="Internal")
    scores = nc.dram_tensor("scores", [rows_per_core, 1], dt.float32, kind="ExternalOutput")

    from contextlib import ExitStack

    with tile.TileContext(nc) as tc, ExitStack() as ctx:
        const = ctx.enter_context(tc.tile_pool(name="const", bufs=1))

        def load_const(name, shape, dtype):
            t = const.tile(shape, dtype, tag=f"const_{name}")
            nc.sync.dma_start(t[:], dr[name].ap())
            return t

        wsb = []
        for i in range(L):
            wsb.append(
                (
                    load_const(f"wa{i}", [P, 4 * D], wdt),
                    load_const(f"w1_{i}", [P, 4 * HID], wdt),
                    load_const(f"w2_{i}", [P, 2 * D], wdt),
                )
            )
        h1sb = load_const("h1", [P, 8 * HID], dt.bfloat16)
        h2sb = load_const("h2", [P, 2], dt.bfloat16)
        q0T = load_const("q0T", [P, 4 * R], adt)
        identb = load_const("identb", [P, P], wdt)
        omat = load_const("omat", [P, P], dt.bfloat16)
        eps_t = const.tile([P, 1], dt.float32, tag="eps")
        nc.gpsimd.memset(eps_t[:], float(EPS))

        xp = ctx.enter_context(tc.tile_pool(name="xp", bufs=4))
        zp = ctx.enter_context(tc.tile_pool(name="zp", bufs=4))
        sqp = ctx.enter_context(tc.tile_pool(name="sqp", bufs=3))
        stp = ctx.enter_context(tc.tile_pool(name="stp", bufs=5))
        ap_ = ctx.enter_context(tc.tile_pool(name="ap", bufs=18))
        hp = ctx.enter_context(tc.tile_pool(name="hp", bufs=4))
        fin = ctx.enter_context(tc.tile_pool(name="fin", bufs=1))
        lout = ctx.enter_context(tc.tile_pool(name="lout", bufs=2))
        py = ctx.enter_context(tc.tile_pool(name="py", bufs=5, space="PSUM"))
        pst = ctx.enter_context(tc.tile_pool(name="pst", bufs=2, space="PSUM"))
        plg = ctx.enter_context(tc.tile_pool(name="plg", bufs=1, space="PSUM"))

        # residual chunks 0..PE_RES-1 added via identity matmul in PSUM;
        # chunks PE_RES..3 added by DVE during PSUM evacuation.
        PE_RES = int(os.environ.get("KERNEL_PE_RES", "2"))

        def pair2(ap_in, stride, num):
            """[128, X] AP -> [128, 2, num] AP pairing two adjacent k-tiles."""
            return bass.AP(
                ap_in.tensor, ap_in.offset, [ap_in.ap[0], [stride, 2], [1, num]]
            )

        def mm_stage(w_sb, rhs, nk, nfo, resid=None, force_bf16=False):
            """y^T chunks: out[fo] = sum_k w[k,fo-block]^T @ rhs[k] (+ resid[fo]).

            w_sb: [128, nk*nfo*128] chunked lhsT; rhs(k) -> [128, R] AP;
            resid(fo) -> [128, R] AP or None (PE-added only for fo < PE_RES).
            Under fp8, k-chunks are processed in pairs with DoubleRow perf
            mode (256-deep contraction per matmul at double rate).
            Returns list of PSUM tiles.
            """
            m = nfo * P
            ys = []
            for fo in range(nfo):
                pe_resid = resid is not None and fo < PE_RES
                y = py.tile([P, R], dt.float32, tag="y")
                if FP8 and not force_bf16:
                    assert nk % 2 == 0
                    for t in range(nk // 2):
                        w_ap = w_sb[:, 2 * t * m + fo * P : 2 * t * m + fo * P + P]
                        r_ap = rhs(2 * t)
                        nc.tensor.matmul(
                            y[:, :],
                            pair2(w_ap, m, P),
                            pair2(r_ap, R, R),
                            start=(t == 0),
                            stop=(t == nk // 2 - 1 and not pe_resid),
                            perf_mode=mybir.MatmulPerfMode.DoubleRow,
                        )
                else:
                    for k in range(nk):
                        nc.tensor.matmul(
                            y[:, :],
                            w_sb[:, k * m + fo * P : k * m + (fo + 1) * P],
                            rhs(k),
                            start=(k == 0),
                            stop=(k == nk - 1 and not pe_resid),
                        )
                if pe_resid:
                    nc.tensor.matmul(
                        y[:, :], identb[:], resid(fo), start=False, stop=True
                    )
                ys.append(y)
            return ys

        def ln_multi(ys_list, resid_list, out_dtype=None, twin=False,
                     need_mean=False):
            """LN for a wave of in-flight blocks; ops interleaved op-type-major
            across blocks so no engine stream has long dependent runs.

            Chunks >= PE_RES get their residual added during the DVE
            evacuation; chunks < PE_RES were already summed in PSUM.

            need_mean=False: the LN input is zero-mean by construction
            (host-centered weights + LN-output residuals), so the variance is
            just the second moment and no mean subtraction happens at all.
            need_mean=True (only layer-0 c-side, whose residual is the raw
            candidate embedding): full mean/variance path."""
            n = len(ys_list)
            Z, SQ, SBZ, SBQ, ME, MU2, VEPS, STD, RSTD, A = (
                [None] * n for _ in range(10)
            )
            for j, ys in enumerate(ys_list):
                _lvl = int(os.environ.get("KERNEL_LN_LEVEL", "3"))
                zpool = ap_ if _lvl < 3 else zp
                Z[j] = zpool.tile(
                    [P, 4 * R], adt if _lvl < 3 else dt.bfloat16,
                    name=f"z{j}", tag="z",
                )
                for c in range(4):
                    if c < PE_RES:
                        # psum holds WS*(W x + resid); evac unscales
                        nc.scalar.activation(
                            out=Z[j][:, ts(c, R)], in_=ys[c][:], func=act_fn.Copy,
                            scale=1.0 / WS if FP8 else 1.0,
                        )
                    elif FP8:
                        nc.vector.scalar_tensor_tensor(
                            out=Z[j][:, ts(c, R)], in0=ys[c][:],
                            scalar=1.0 / WS, in1=resid_list[j](c),
                            op0=alu.mult, op1=alu.add,
                        )
                    else:
                        nc.vector.tensor_tensor(
                            out=Z[j][:, ts(c, R)], in0=ys[c][:],
                            in1=resid_list[j](c), op=alu.add,
                        )
            LVL = int(os.environ.get("KERNEL_LN_LEVEL", "3"))
            NOSQ = os.environ.get("KERNEL_NOSQ", "0") == "1"
            NOSTATS = os.environ.get("KERNEL_NOSTATS", "0") == "1"
            NORECIP = os.environ.get("KERNEL_NORECIP", "0") == "1"
            NB = os.environ.get("KERNEL_NB", "0") == "1"
            if LVL == 0:
                return (Z, Z) if twin else Z
            # z^2 on DVE (bf16 SBUF-only tensor_tensor runs packed and is the
            # cheapest bulk elementwise path on HW; Act was the loaded engine
            # after the zero-mean restructure removed its other LN work).
            SQ_SPLIT = os.environ.get("KERNEL_SQ_SPLIT", "1") == "1"
            for j in range(n):
                SQ[j] = sqp.tile([P, 4 * R], dt.bfloat16, name=f"zsq{j}", tag="zsq")
                if SQ_SPLIT:
                    nc.scalar.activation(
                        out=SQ[j][:, 0 : 2 * R], in_=Z[j][:, 0 : 2 * R],
                        func=act_fn.Square,
                    )
                    nc.vector.tensor_tensor(
                        out=SQ[j][:, 2 * R : 4 * R], in0=Z[j][:, 2 * R : 4 * R],
                        in1=Z[j][:, 2 * R : 4 * R], op=alu.mult,
                    )
                else:
                    for h in range(2):
                        nc.vector.tensor_tensor(
                            out=SQ[j][:, h * 2 * R : (h + 1) * 2 * R],
                            in0=Z[j][:, h * 2 * R : (h + 1) * 2 * R],
                            in1=Z[j][:, h * 2 * R : (h + 1) * 2 * R],
                            op=alu.mult,
                        )
            # stats: sum + /512 + partition-broadcast in one PE op per chunk:
            # SBQ = (ones/512)^T @ sq-chunks, accumulated -> every partition
            # holds the column mean of z^2 (the variance, since z is
            # zero-mean unless need_mean).  The sum-z pass exists only for
            # the need_mean LN.
            if need_mean:
                for j in range(n):
                    SBZ[j] = py.tile([P, R], dt.float32, name=f"sbz{j}", tag="y")
                    for c in range(4):
                        nc.tensor.matmul(
                            SBZ[j][:], omat[:], Z[j][:, ts(c, R)],
                            start=(c == 0), stop=(c == 3),
                        )
            for j in range(n):
                SBQ[j] = pst.tile([P, R], dt.float32, name=f"sbq{j}", tag="sb")
                for c in range(4):
                    nc.tensor.matmul(
                        SBQ[j][:], omat[:], SQ[j][:, ts(c, R)],
                        start=(c == 0), stop=(c == 3),
                    )
            if need_mean:
                # ME and MU2 depend only on SBZ (done before the sumsq
                # matmuls), so both run during the sumsq pass.
                for j in range(n):
                    ME[j] = stp.tile([P, R], dt.bfloat16, name=f"me{j}", tag="me")
                    nc.scalar.activation(
                        out=ME[j][:], in_=SBZ[j][:], func=act_fn.Copy
                    )
            if LVL == 1:
                return (Z, Z) if twin else Z
            if need_mean:
                for j in range(n):
                    MU2[j] = stp.tile([P, R], dt.bfloat16, name=f"mu2{j}", tag="mu2")
                    nc.vector.tensor_tensor(
                        out=MU2[j][:], in0=ME[j][:], in1=ME[j][:], op=alu.mult
                    )
                for j in range(n):
                    VEPS[j] = stp.tile(
                        [P, R], dt.bfloat16, name=f"veps{j}", tag="veps"
                    )
                    nc.vector.scalar_tensor_tensor(
                        out=VEPS[j][:], in0=SBQ[j][:], scalar=1.0,
                        in1=MU2[j][:], op0=alu.bypass, op1=alu.subtract,
                    )
            for j in range(n):
                STD[j] = stp.tile([P, R], dt.float32, name=f"std{j}", tag="std")
                nc.scalar.activation(
                    out=STD[j][:],
                    in_=VEPS[j][:] if need_mean else SBQ[j][:],
                    func=act_fn.Sqrt, bias=eps_t[:],
                )

            if LVL == 2:
                return (Z, Z) if twin else Z

            for j in range(n):
                A[j] = ap_.tile(
                    [P, 4 * R], out_dtype or adt, name=f"a{j}", tag="a"
                )
            # normalize per chunk with plain 2-level stride-1 APs (stride-0
            # broadcast APs measured ~3x slower on HW); the [128,R] stat tile
            # already holds the per-column value on every partition.  Under
            # fp8 the subtract lands in the (now-free) bf16 SQ tile so the
            # activation is only fp8-quantized once, at the final multiply.
            if need_mean:
                AS = [SQ[j] if FP8 else A[j] for j in range(n)]
                for c in range(4):
                    for j in range(n):
                        nc.vector.tensor_tensor(
                            out=AS[j][:, ts(c, R)],
                            in0=Z[j][:, ts(c, R)],
                            in1=ME[j][:],
                            op=alu.subtract,
                        )
            else:
                AS = Z
            for j in range(n):
                # fp32 1/std via the fast custom-DVE op (InstReciprocal
                # measured ~4.5x slower on HW than the cost model claims),
                # then a bf16 convert on Act so the multiplies stay 2-byte.
                RSTDF = stp.tile([P, R], dt.float32, name=f"rstdf{j}", tag="rstdf")
                nc.vector.reciprocal_approx_fast(out=RSTDF[:], in_=STD[j][:])
                RSTD[j] = stp.tile([P, R], dt.bfloat16, name=f"rstd{j}", tag="rstd")
                nc.scalar.activation(
                    out=RSTD[j][:], in_=RSTDF[:], func=act_fn.Copy
                )
            for c in range(4):
                for j in range(n):
                    nc.vector.tensor_tensor(
                        out=A[j][:, ts(c, R)],
                        in0=AS[j][:, ts(c, R)],
                        in1=RSTD[j][:],
                        op=alu.mult,
                    )
            if not twin:
                return A
            # bf16 twin of the normalized output (for the bf16 head path)
            AB = [None] * n
            for j in range(n):
                AB[j] = ap_.tile(
                    [P, 4 * R], dt.bfloat16, name=f"ab{j}", tag="ab", bufs=WAVE + 1
                )
            for c in range(4):
                for j in range(n):
                    nc.vector.tensor_tensor(
                        out=AB[j][:, ts(c, R)],
                        in0=AS[j][:, ts(c, R)],
                        in1=RSTD[j][:],
                        op=alu.mult,
                    )
            return A, AB

        def input_stage(b):
            cT = xp.tile([P, 4 * R], adt, tag="cT")
            nc.sync.dma_start(cT[:], candT.ap()[:, b * 4 * R : (b + 1) * 4 * R])
            return cT

        def relu_multi(hps_list, out_dtype, tag, unscale=True):
            # relu is positively homogeneous: Relu(psum/WS) unscales exactly
            hs = []
            for j, hps in enumerate(hps_list):
                h = hp.tile([P, 2 * R], out_dtype, name=f"h{j}", tag=tag)
                for fo in range(2):
                    nc.scalar.activation(
                        out=h[:, ts(fo, R)], in_=hps[fo][:], func=act_fn.Relu,
                        scale=1.0 / WS if (FP8 and unscale) else 1.0,
                    )
                hs.append(h)
            return hs

        for w0 in range(0, nblk, WAVE):
            wb = list(range(w0, min(w0 + WAVE, nblk)))
            st = [{"b": b} for b in wb]
            for S in st:
                S["cT"] = input_stage(S["b"])
                S["q"], S["c"] = q0T, S["cT"]
            for i in range(L):
                wa, w1, w2 = wsb[i]
                for S in st:
                    S["y"] = mm_stage(
                        wa, lambda k, S=S: S["c"][:, ts(k, R)], 4, 4,
                        resid=lambda fo, S=S: S["q"][:, ts(fo, R)],
                    )
                a1s = ln_multi(
                    [S["y"] for S in st],
                    [lambda c, S=S: S["q"][:, ts(c, R)] for S in st],
                )
                for S, a1 in zip(st, a1s):
                    S["a1"] = a1
                    S["hps"] = mm_stage(w1, lambda k, a1=a1: a1[:, ts(k, R)], 4, 2)
                hs = relu_multi([S["hps"] for S in st], adt, "h")
                for S, h in zip(st, hs):
                    S["y"] = mm_stage(
                        w2, lambda k, h=h: h[:, ts(k, R)], 2, 4,
                        resid=lambda fo, S=S: S["a1"][:, ts(fo, R)],
                    )
                if FP8 and i == L - 1:
                    # bf16 twin of the final q for the bf16 head path
                    a2s, a2bs = ln_multi(
                        [S["y"] for S in st],
                        [lambda c, S=S: S["a1"][:, ts(c, R)] for S in st],
                        twin=True,
                    )
                else:
                    a2s = ln_multi(
                        [S["y"] for S in st],
                        [lambda c, S=S: S["a1"][:, ts(c, R)] for S in st],
                    )
                    a2bs = a2s
                for S, a2, a2b in zip(st, a2s, a2bs):
                    S["a2"] = a2
                    S["qh"] = a2b
                    S["y"] = mm_stage(
                        wa, lambda k, a2=a2: a2[:, ts(k, R)], 4, 4,
                        resid=lambda fo, S=S: S["c"][:, ts(fo, R)],
                    )
                a3s = ln_multi(
                    [S["y"] for S in st],
                    [lambda c, S=S: S["c"][:, ts(c, R)] for S in st],
                    need_mean=(i == 0),
                )
                for S, a3 in zip(st, a3s):
                    S["a3"] = a3
                    S["hps"] = mm_stage(w1, lambda k, a3=a3: a3[:, ts(k, R)], 4, 2)
                hs = relu_multi([S["hps"] for S in st], adt, "h")
                for S, h in zip(st, hs):
                    S["y"] = mm_stage(
                        w2, lambda k, h=h: h[:, ts(k, R)], 2, 4,
                        resid=lambda fo, S=S: S["a3"][:, ts(fo, R)],
                    )
                a4s = ln_multi(
                    [S["y"] for S in st],
                    [lambda c, S=S: S["a3"][:, ts(c, R)] for S in st],
                    out_dtype=dt.bfloat16 if i == L - 1 else None,
                )
                for S, a4 in zip(st, a4s):
                    S["q"], S["c"] = S["a2"], a4

            # head: combined = [q | c] -> HID -> 1, fully bf16 (head errors
            # reach the logits unwashed by any later LN)
            for S in st:
                S["hps"] = mm_stage(
                    h1sb,
                    lambda k, S=S: (
                        S["qh"][:, ts(k, R)] if k < 4 else S["c"][:, ts(k - 4, R)]
                    ),
                    8, 2, force_bf16=True,
                )
            hhs = relu_multi([S["hps"] for S in st], dt.bfloat16, "hh", unscale=False)
            for S, hh in zip(st, hhs):
                lg = plg.tile([1, R], dt.float32, tag="lg")
                for k in range(2):
                    nc.tensor.matmul(
                        lg[:, :], h2sb[:, k : k + 1], hh[:, ts(k, R)],
                        start=(k == 0), stop=(k == 1),
                    )
                lgo = lout.tile([1, R], dt.float32, tag="lgo")
                nc.scalar.activation(out=lgo[:], in_=lg[:], func=act_fn.Copy)
                nc.sync.dma_start(
                    lgs.ap().rearrange("(b j) -> b j", j=R)[S["b"] : S["b"] + 1, :],
                    lgo[:],
                )

        # tail: logits -> sigmoid -> scores (one ACT table switch total)
        jpp = (nblk * R) // P  # logits per partition
        lsb = fin.tile([P, jpp], dt.float32, tag="lsb")
        nc.sync.dma_start(lsb[:], lgs.ap().rearrange("(p j) -> p j", j=jpp))
        sig = fin.tile([P, jpp], dt.float32, tag="sig")
        nc.scalar.activation(out=sig[:], in_=lsb[:], func=act_fn.Sigmoid)
        nc.sync.dma_start(
            scores.ap().rearrange("(p j) o -> p (j o)", j=jpp), sig[:]
        )

    nc.compile()
    return nc


def _get_program(rows_per_core: int):
    if rows_per_core not in _cache:
        _cache[rows_per_core] = _build_program(rows_per_core)
    return _cache[rows_per_core]


def _build_runner(nc, n_cores):
    """Cached jitted PJRT runner (mirror of bass2jax.run_bass_via_pjrt without
    donation, so device args can be reused across calls)."""
    import jax
    from jax.sharding import Mesh, PartitionSpec
    from jax.experimental.shard_map import shard_map
    import concourse.mybir as mybir
    from concourse.bass2jax import (
        install_neuronx_cc_hook,
        partition_id_tensor,
        _bass_exec_p,
    )

    install_neuronx_cc_hook()
    partition_name = nc.partition_id_tensor.name if nc.partition_id_tensor else None

    in_names, out_names, out_avals, zero_outs = [], [], [], []
    for alloc in nc.m.functions[0].allocations:
        if not isinstance(alloc, mybir.MemoryLocationSet):
            continue
        name = alloc.memorylocations[0].name
        if alloc.kind == "ExternalInput":
            if name != partition_name:
                in_names.append(name)
        elif alloc.kind == "ExternalOutput":
            out_names.append(name)
            shape = tuple(alloc.tensor_shape)
            dtype = mybir.dt.np(alloc.dtype)
            out_avals.append(jax.core.ShapedArray(shape, dtype))
            zero_outs.append(np.zeros(shape, dtype))
    n_params = len(in_names)
    all_in_names = list(in_names) + list(out_names)
    if partition_name is not None:
        all_in_names.append(partition_name)

    def _body(*args):
        operands = list(args)
        if partition_name is not None:
            operands.append(partition_id_tensor())
        outs = _bass_exec_p.bind(
            *operands,
            out_avals=tuple(out_avals),
            in_names=tuple(all_in_names),
            out_names=tuple(out_names),
            lowering_input_output_aliases=(),
            sim_require_finite=True,
            sim_require_nnan=True,
            nc=nc,
        )
        return tuple(outs)

    devices = jax.devices()[:n_cores]
    assert len(devices) == n_cores
    mesh = Mesh(np.asarray(devices), ("core",))
    n_outs = len(out_names)
    fn = jax.jit(
        shard_map(
            _body, mesh=mesh,
            in_specs=(PartitionSpec("core"),) * (n_params + n_outs),
            out_specs=(PartitionSpec("core"),) * n_outs,
            check_rep=False,
        ),
        keep_unused=True,
    )
    return fn, mesh, in_names, out_names, zero_outs


def kernel(**inputs) -> np.ndarray:
    import jax
    from jax.sharding import NamedSharding, PartitionSpec

    arrs = _prep_host(inputs)
    candT = _prep_cand(inputs["candidate_embeddings"])  # [NCORES, P, nblk*4*R]
    n = np.asarray(inputs["candidate_embeddings"]).shape[0]
    rows_per_core = n // NCORES
    nc = _get_program(rows_per_core)

    if rows_per_core not in _runner_cache:
        _runner_cache[rows_per_core] = _build_runner(nc, NCORES)
    fn, mesh, in_names, out_names, zero_outs = _runner_cache[rows_per_core]

    per_core_named = {"candT": candT}  # [NCORES, ...] stacked
    concat_in = []
    for nm in in_names:
        if nm in per_core_named:
            a = per_core_named[nm].reshape(-1, *per_core_named[nm].shape[2:])
        else:
            a = np.concatenate([arrs[nm]] * NCORES, axis=0)
        concat_in.append(a)
    concat_zeros = [
        np.zeros((NCORES * z.shape[0], *z.shape[1:]), z.dtype) for z in zero_outs
    ]
    sh = NamedSharding(mesh, PartitionSpec("core"))
    dev_args = [jax.device_put(a, sh) for a in concat_in + concat_zeros]
    outs = fn(*dev_args)
    out_map = {nm: np.asarray(o) for nm, o in zip(out_names, outs)}
    return out_map["scores"].reshape(n, 1).astype(np.float32)


if __name__ == "__main__":
    rows = int(sys.argv[1]) if len(sys.argv) > 1 else 512
    nc = _build_program(rows)
    print("built ok:", rows)
